# revision 5
# baseline (speedup 1.0000x reference)
"""CPSF fused codebook kernel for Trainium2 (8 NeuronCores, codebook-parallel).

Sharding: M (codebook, 4096) split 8 ways -> 512 entries/core; every core sees
all B=2048 queries. Host sums the 8 partial [B,S] outputs.

Math (per m-tile j of 128 codebook entries):
  x = djx^T z, y = djy^T z                      (PE, f32r)
  F3 = f3z^T z + f3d^T d + f3c8^T rhsc8        (PE, f32r/bf16; holds c_o*q0,
                                                pconst, angular dist, log-alpha)
  in2 = Gd*(x^2+y^2) + F3                      (ACT squares + DVE add/STT)
  S   = sum_k exp(u'_k x + v'_k)               (8 ACT exps -> bf16 slabs,
                                                DVE/GPSIMD add tree)
  wgt = exp(in2) * S                           (ACT exp + DVE mul, bf16)
  T  += That^T wgt                             (PE, bf16)

ACT is the bottleneck engine (9 exps + 2 squares per (m-tile, query) are
algorithmically required); emission order keeps its queue streaming:
Sqx, E0, Sqy0, E1, Sqy1, E2, Sqy2, E3, E4, Sqy3, E5..E7, EB per tile.
"""

import numpy as np
import ml_dtypes

B, M, N, S, K = 2048, 4096, 64, 64, 8
EPS = 1e-3
NCORES = 8
ML = M // NCORES          # 512 codebook entries per core
MT = ML // 128            # 4 m-tiles per core
NQ = 4                    # b-quarters (PSUM-sized chunks of 512)
BQ = B // NQ              # 512
f32 = np.float32
bf16 = ml_dtypes.bfloat16

_CACHE = {}


def _bf16_rt(x):
    u = np.ascontiguousarray(x, np.float32).view(np.uint32)
    r = (u + 0x7FFF + ((u >> 16) & 1)) & 0xFFFF0000
    return r.view(np.float32).astype(np.float64)


def _prep(z_re, z_im, d_re, d_im, zj_re, zj_im, dj_re, dj_im,
          That_re, That_im, alpha, sig_par, sig_perp):
    """Host-side packing: fp64 exact, cast to fp32/bf16 at the end."""
    x64 = lambda a: np.asarray(a, np.float64)
    zr, zi, dr, di = map(x64, (z_re, z_im, d_re, d_im))
    zjr, zji, djr, dji = map(x64, (zj_re, zj_im, dj_re, dj_im))

    tgl, wgl = np.polynomial.legendre.leggauss(K)
    t = (0.5 * (tgl + 1.0)).astype(f32).astype(np.float64)
    wq = (0.5 * wgl).astype(f32).astype(np.float64)

    dd2 = (djr**2 + dji**2).sum(-1)                          # [M]
    c_re = (djr * zjr + dji * zji).sum(-1)
    c_im = (djr * zji - dji * zjr).sum(-1)
    sp2 = x64(sig_par)**2 + EPS
    so2 = x64(sig_perp)**2 + EPS
    G = -0.5 / sp2
    c_o = -0.5 / so2
    Gd = G - c_o
    umid = -G * dd2
    lnal = np.log(np.maximum(x64(alpha), 1e-38))
    nzj = (zjr**2 + zji**2).sum(-1)
    nz = (zr**2 + zi**2).sum(-1)                             # [B]
    nd = (dr**2 + di**2).sum(-1)

    u = np.stack([-2.0 * G * t[k] * dd2 for k in range(K)])  # [K,M]
    up = u - umid[None, :]
    vp = np.stack([np.log(wq[k]) + G * (t[k] * dd2)**2 - up[k] * c_re
                   for k in range(K)])

    djx = np.concatenate([djr.T, dji.T], 0)                  # [128, M]
    djy = np.concatenate([-dji.T, djr.T], 0)
    f3z = ((-2.0 * c_o) * np.concatenate([zjr.T, zji.T], 0)
           + (-2.0 * Gd * c_re + umid) * djx
           + (-2.0 * Gd * c_im) * djy)
    f3d = (-2.0 * c_o) * np.concatenate([djr.T, dji.T], 0)
    const0 = (c_o * (nzj + dd2) + Gd * (c_re**2 + c_im**2)
              + lnal - umid * c_re)

    # f3c8/rhsc8: rank-8 bf16-exact update for co*(nz+nd) + pconst
    co_h = _bf16_rt(c_o)
    co_l = c_o - co_h
    pc_h = _bf16_rt(const0)
    pc_l = const0 - pc_h
    nz_h = _bf16_rt(nz)
    nz_l = nz - nz_h
    nd_h = _bf16_rt(nd)
    nd_l = nd - nd_h
    ones = np.ones_like(nz)
    f3c8 = np.stack([co_h, co_h, co_l, co_h, co_h, co_l, pc_h, pc_l])
    rhsc8 = np.stack([nz_h, nz_l, nz_h, nd_h, nd_l, nd_h, ones, ones])
    that2 = np.concatenate([x64(That_re), x64(That_im)], 1)  # [M, 128]

    # pk: per m-tile block [djx | djy | f3z], each 128 cols (f32r path)
    nt = M // 128
    pk = np.empty((128, nt * 384), np.float64)
    for j in range(nt):
        ms = slice(j * 128, (j + 1) * 128)
        pk[:, j * 384 + 0:j * 384 + 128] = djx[:, ms]
        pk[:, j * 384 + 128:j * 384 + 256] = djy[:, ms]
        pk[:, j * 384 + 256:j * 384 + 384] = f3z[:, ms]

    pGd = Gd.reshape(nt, 128).T                              # [128, nt]
    pu = up.reshape(K, nt, 128).transpose(2, 0, 1).reshape(128, K * nt)
    pv = vp.reshape(K, nt, 128).transpose(2, 0, 1).reshape(128, K * nt)

    c = lambda a: np.ascontiguousarray(a, dtype=f32)
    cb = lambda a: np.ascontiguousarray(np.asarray(a, np.float32), dtype=bf16)
    return dict(pk=c(pk), f3c8=cb(f3c8), rhsc8=cb(rhsc8), that2=cb(that2),
                f3db=cb(f3d), pGd=c(pGd), pu=c(pu), pv=c(pv),
                zst=c(np.concatenate([zr.T, zi.T], 0)),
                dstb=cb(np.concatenate([dr.T, di.T], 0)))


def _core_slices(p, cid):
    """Per-core in_map from the full packed arrays (m-sharded)."""
    jt = slice(cid * MT * 384, (cid + 1) * MT * 384)         # pk cols
    ms = slice(cid * ML, (cid + 1) * ML)
    jc = slice(cid * MT, (cid + 1) * MT)
    kc = np.concatenate([np.arange(k * (M // 128) + cid * MT,
                                   k * (M // 128) + (cid + 1) * MT)
                         for k in range(K)])
    cc = np.ascontiguousarray
    return {"pk": cc(p["pk"][:, jt]), "f3c8": cc(p["f3c8"][:, ms]),
            "that2": cc(p["that2"][ms, :]), "pGd": cc(p["pGd"][:, jc]),
            "f3db": cc(p["f3db"][:, ms]),
            "pu": cc(p["pu"][:, kc]), "pv": cc(p["pv"][:, kc]),
            "zst": p["zst"], "dstb": p["dstb"], "rhsc8": p["rhsc8"]}


def _device_maps(maps):
    dev_maps = []
    for m in maps:
        dm = {k: m[k] for k in ("pk", "f3c8", "that2", "zst", "dstb",
                                "rhsc8", "f3db")}
        dm["params"] = np.ascontiguousarray(np.concatenate(
            [m["pGd"], m["pu"], m["pv"]], axis=1))
        dev_maps.append(dm)
    return dev_maps


def _emulate_core(m):
    """Numpy emulation of one core's device program."""
    zst = np.asarray(m["zst"], f32)
    dstb = np.asarray(m["dstb"], np.float32)
    rhsc8 = np.asarray(m["rhsc8"], np.float32)
    that2 = np.asarray(m["that2"], np.float32)
    f3db = np.asarray(m["f3db"], np.float32)
    t_acc = np.zeros((128, 2048), f32)
    for j in range(MT):
        pkj = m["pk"][:, j * 384:(j + 1) * 384]
        djx_t, djy_t, f3z_t = pkj[:, 0:128], pkj[:, 128:256], pkj[:, 256:384]
        f3d_t = f3db[:, j * 128:(j + 1) * 128]
        x = (djx_t.T @ zst).astype(f32)
        y = (djy_t.T @ zst).astype(f32)
        F3 = (f3z_t.T @ zst + f3d_t.T @ dstb
              + m["f3c8"][:, j * 128:(j + 1) * 128].astype(np.float32).T
              @ rhsc8).astype(f32)
        s = (x * x + y * y).astype(f32)
        in2 = (s * m["pGd"][:, j:j + 1] + F3).astype(f32)
        Ssum = np.zeros_like(x)
        for k in range(K):
            col = k * MT + j
            arg = (x * m["pu"][:, col:col + 1] + m["pv"][:, col:col + 1]).astype(f32)
            Ssum = (Ssum + np.exp(arg, dtype=f32)).astype(f32)
        wgt = (np.exp(in2, dtype=f32) * Ssum).astype(f32)
        that_t = that2[j * 128:(j + 1) * 128, :]
        t_acc += (that_t.T @ wgt).astype(f32)
    return t_acc


def _build_bass():
    import concourse.bacc as bacc
    import concourse.mybir as mybir
    from concourse import tile

    dt = mybir.dt.float32
    f32r = mybir.dt.float32r
    bfdt = mybir.dt.bfloat16
    AF = mybir.ActivationFunctionType
    AO = mybir.AluOpType
    nc = bacc.Bacc("TRN2", target_bir_lowering=False, debug=False)

    dram = {}
    for name, shape, dty in [("zst", [128, B], f32r),
                             ("dstb", [128, B], bfdt),
                             ("rhsc8", [8, B], bfdt),
                             ("pk", [128, MT * 384], f32r),
                             ("f3c8", [8, ML], bfdt),
                             ("f3db", [128, ML], bfdt),
                             ("that2", [ML, 128], bfdt),
                             ("params", [128, MT + 2 * K * MT], dt)]:
        dram[name] = nc.dram_tensor(name, shape, dty, kind="ExternalInput")
    tout = nc.dram_tensor("tout", [128, B], dt, kind="ExternalOutput")

    HB = B // 2
    with tile.TileContext(nc) as tc:
        with tc.tile_pool(name="const", bufs=1) as cpool:
            params = cpool.tile([128, MT + 2 * K * MT], dt)
            pGd = params[:, 0:MT]
            pu = params[:, MT:MT + K * MT]
            pv = params[:, MT + K * MT:MT + 2 * K * MT]
            rhsc8 = cpool.tile([8, B], bfdt)
            that_all = cpool.tile([128, MT * 128], bfdt)
            f3c8_all = cpool.tile([8, ML], bfdt)
            f3db_all = cpool.tile([128, ML], bfdt)
            warm = cpool.tile([128, 8], dt)
            zqs = [cpool.tile([128, BQ], f32r, name=f"zq{i}") for i in range(NQ)]
            dqs = cpool.tile([128, B], bfdt)

            with (
                tc.tile_pool(name="lhs", bufs=2) as lpool,
                tc.tile_pool(name="work", bufs=1) as wpool,
                tc.tile_pool(name="eslab", bufs=2) as epool,
                tc.tile_pool(name="wgtp", bufs=1) as gpool,
            ):
                # DMA split across queues; critical path (zq, pk) on sync
                pk0x = lpool.tile([128, 128], f32r, tag="pkx")
                pk0 = lpool.tile([128, 384], f32r, tag="pk")
                nc.sync.dma_start(pk0x[:, :], dram["pk"][:, 0:128])
                nc.sync.dma_start(zqs[0][:, :], dram["zst"][:, 0:BQ])
                # warm the exp table as soon as the first DMA lands
                nc.scalar.activation(warm[:, :], pk0x[:, 0:8], AF.Exp)
                nc.sync.dma_start(zqs[1][:, :], dram["zst"][:, BQ:2 * BQ])
                nc.scalar.dma_start(zqs[2][:, :], dram["zst"][:, 2 * BQ:3 * BQ])
                nc.scalar.dma_start(zqs[3][:, :], dram["zst"][:, 3 * BQ:4 * BQ])
                nc.sync.dma_start(pk0[:, :], dram["pk"][:, 0:384])
                nc.gpsimd.dma_start(params[:, :], dram["params"][:, :])
                nc.gpsimd.dma_start(dqs[:, :], dram["dstb"][:, :])
                nc.gpsimd.dma_start(rhsc8[:, :], dram["rhsc8"][:, :])
                nc.gpsimd.dma_start(f3c8_all[:, :], dram["f3c8"][:, :])
                nc.gpsimd.dma_start(f3db_all[:, :], dram["f3db"][:, :])
                nc.gpsimd.dma_start(
                    that_all[:, :].rearrange("p (j c) -> p j c", j=MT),
                    dram["that2"][:, :].rearrange("(j p) c -> p j c", p=128))
                wgts = []
                with (
                    tc.tile_pool(name="xps", bufs=1, space="PSUM") as xpool,
                    tc.tile_pool(name="yf", bufs=2, space="PSUM") as qpool,
                    tc.tile_pool(name="tp0", bufs=1, space="PSUM") as tpool0,
                ):
                    tph0 = tpool0.tile([128, HB], dt, tag="tph0")
                    for j in range(MT):
                        if j == 0:
                            pk_t, pkx_t = pk0, pk0x
                        else:
                            pk_t = lpool.tile([128, 384], f32r, tag="pk")
                            pkx_t = lpool.tile([128, 128], f32r, tag="pkx")
                            nc.sync.dma_start(
                                pkx_t[:, :],
                                dram["pk"][:, j * 384:j * 384 + 128])
                            nc.sync.dma_start(
                                pk_t[:, :],
                                dram["pk"][:, j * 384:(j + 1) * 384])
                        djx_t = pkx_t[:, :]
                        djy_t = pk_t[:, 128:256]
                        f3z_t = pk_t[:, 256:384]
                        f3c8_t = f3c8_all[:, j * 128:(j + 1) * 128]
                        f3d_t = f3db_all[:, j * 128:(j + 1) * 128]

                        xx = wpool.tile([128, B], dt, tag="xx")
                        yy = wpool.tile([128, B], dt, tag="yy")
                        ss = wpool.tile([128, B], dt, tag="ss")
                        in2 = wpool.tile([128, B], dt, tag="in2")
                        EB = wpool.tile([128, B], bfdt, tag="EB")
                        slabs = [epool.tile([128, 2 * B], bfdt, tag=f"esl{p}",
                                            name=f"esl{p}_{j}")
                                 for p in range(4)]
                        sa = wpool.tile([128, B], bfdt, tag="sa")
                        sb = wpool.tile([128, B], bfdt, tag="sb")
                        sc = wpool.tile([128, B], bfdt, tag="sc")
                        sd = wpool.tile([128, B], bfdt, tag="sd")
                        wgt = gpool.tile([128, B], bfdt, tag=f"wgt{j}")

                        x_ps = xpool.tile([128, B], dt, tag="x")
                        for q in range(NQ):
                            nc.tensor.matmul(x_ps[:, q * BQ:(q + 1) * BQ],
                                             djx_t, zqs[q][:, :],
                                             start=True, stop=True)
                        if j > 0:
                            that_p = that_all[:, (j - 1) * 128:j * 128]
                            for q2 in range(2):
                                qs2 = slice(q2 * BQ, (q2 + 1) * BQ)
                                nc.tensor.matmul(tph0[:, qs2], that_p,
                                                 wgts[j - 1][:, qs2],
                                                 start=(j - 1 == 0),
                                                 stop=False)

                        def E(k):
                            col = k * MT + j
                            nc.scalar.activation(
                                slabs[k // 2][:, (k % 2) * B:(k % 2 + 1) * B],
                                x_ps[:, :], AF.Exp,
                                bias=pv[:, col:col + 1],
                                scale=pu[:, col:col + 1])

                        def YFpe(q):
                            """PE part: y matmul, then F3 accumulation."""
                            qs = slice(q * BQ, (q + 1) * BQ)
                            yf = qpool.tile([128, BQ], dt, tag="yf",
                                            name=f"yf{j}_{q}")
                            nc.tensor.matmul(yf[:, :], djy_t, zqs[q][:, :],
                                             start=True, stop=True)
                            return yf, qs

                        def SQY(yf, qs):
                            nc.scalar.activation(yy[:, qs], yf[:, :],
                                                 AF.Square)

                        def F3pe(yf, qs, q):
                            nc.tensor.matmul(yf[:, :], f3z_t, zqs[q][:, :],
                                             start=True, stop=False)
                            nc.tensor.matmul(yf[:, :], f3d_t,
                                             dqs[:, qs],
                                             start=False, stop=False)
                            nc.tensor.matmul(yf[:, :], f3c8_t, rhsc8[:, qs],
                                             start=False, stop=True)

                        def BASE(yf, qs):
                            nc.vector.tensor_add(ss[:, qs], xx[:, qs],
                                                 yy[:, qs])
                            nc.vector.scalar_tensor_tensor(
                                in2[:, qs], ss[:, qs], pGd[:, j:j + 1],
                                yf[:, :], AO.mult, AO.add)

                        SL = lambda p, i: slabs[p][:, i * B:(i + 1) * B]

                        # ACT queue order (strict FIFO):
                        # Sqx, E0, Sqy0, E1, Sqy1, E2, Sqy2, E3, E4, Sqy3,
                        # E5, E6, E7, EB -- keeps ACT streaming, frees x_ps
                        # late (E7) but EB's input (in2) ready before E7 ends.
                        nc.scalar.activation(xx[:, :], x_ps[:, :], AF.Square)
                        yf0, qs0 = YFpe(0)
                        E(0)
                        SQY(yf0, qs0)
                        yf1, qs1 = YFpe(1)
                        F3pe(yf0, qs0, 0)
                        E(1)
                        SQY(yf1, qs1)
                        BASE(yf0, qs0)
                        F3pe(yf1, qs1, 1)
                        E(2)
                        yf2, qs2 = YFpe(2)
                        SQY(yf2, qs2)
                        BASE(yf1, qs1)
                        F3pe(yf2, qs2, 2)
                        E(3)
                        E(4)
                        yf3, qs3 = YFpe(3)
                        SQY(yf3, qs3)
                        BASE(yf2, qs2)
                        F3pe(yf3, qs3, 3)
                        E(5)
                        E(6)
                        BASE(yf3, qs3)
                        E(7)
                        nc.vector.tensor_add(sa[:, :], SL(0, 0), SL(0, 1))
                        nc.vector.tensor_add(sb[:, :], SL(1, 0), SL(1, 1))
                        nc.gpsimd.tensor_add(sc[:, :], SL(2, 0), SL(2, 1))
                        nc.gpsimd.tensor_add(sd[:, :], SL(3, 0), SL(3, 1))
                        nc.scalar.activation(EB[:, :], in2[:, :], AF.Exp)
                        nc.vector.tensor_add(sa[:, :], sa[:, :], sb[:, :])
                        nc.vector.tensor_add(sc[:, :], sc[:, :], sd[:, :])
                        nc.vector.tensor_add(sa[:, :], sa[:, :], sc[:, :])
                        nc.vector.tensor_mul(wgt[:, :], EB[:, :], sa[:, :])
                        wgts.append(wgt)

                    ocp = wpool.tile([128, B], dt, tag="ocp")
                    that_l = that_all[:, (MT - 1) * 128:MT * 128]
                    for q2 in range(2):
                        qs2 = slice(q2 * BQ, (q2 + 1) * BQ)
                        nc.tensor.matmul(tph0[:, qs2], that_l,
                                         wgts[MT - 1][:, qs2],
                                         start=False, stop=True)
                        nc.vector.tensor_copy(ocp[:, qs2], tph0[:, qs2])
                        nc.sync.dma_start(tout[:, qs2], ocp[:, qs2])

                with tc.tile_pool(name="tp1", bufs=1, space="PSUM") as tpool1:
                    tph1 = tpool1.tile([128, HB], dt, tag="tph1")
                    for q2 in range(2):
                        qg = 2 + q2
                        qs = slice(qg * BQ, (qg + 1) * BQ)
                        qs2 = slice(q2 * BQ, (q2 + 1) * BQ)
                        for j in range(MT):
                            that_j = that_all[:, j * 128:(j + 1) * 128]
                            nc.tensor.matmul(tph1[:, qs2], that_j,
                                             wgts[j][:, qs],
                                             start=(j == 0),
                                             stop=(j == MT - 1))
                        nc.vector.tensor_copy(ocp[:, qs], tph1[:, qs2])
                        nc.sync.dma_start(tout[:, qs], ocp[:, qs])

    nc.compile()
    return nc


def kernel(z_re, z_im, d_re, d_im, zj_re, zj_im, dj_re, dj_im,
           That_re, That_im, alpha, sig_par, sig_perp, _emulate=False):
    p = _prep(z_re, z_im, d_re, d_im, zj_re, zj_im, dj_re, dj_im,
              That_re, That_im, alpha, sig_par, sig_perp)
    maps = [_core_slices(p, c) for c in range(NCORES)]

    if _emulate:
        outs = [_emulate_core(m) for m in maps]
    else:
        from concourse.bass_utils import run_bass_kernel_spmd
        if "nc" not in _CACHE:
            _CACHE["nc"] = _build_bass()
        dev_maps = _device_maps(maps)
        res = run_bass_kernel_spmd(_CACHE["nc"], dev_maps,
                                   core_ids=list(range(NCORES)))
        outs = [res.results[c]["tout"] for c in range(NCORES)]

    full = np.zeros((128, B), np.float64)
    for o in outs:
        full += o.astype(np.float64)
    full = full.astype(f32).T                   # [B, 128]
    return (full[:, :S] + 1j * full[:, S:]).astype(np.complex64)


# revision 6
# speedup vs baseline: 1.0377x; 1.0377x over previous
"""CPSF fused codebook kernel for Trainium2 (8 NeuronCores, codebook-parallel).

Sharding: M (codebook, 4096) split 8 ways -> 512 entries/core; every core sees
all B=2048 queries. Host sums the 8 partial [B,S] outputs.

Math (per m-tile j of 128 codebook entries):
  x = djx^T z, y = djy^T z                      (PE, f32r)
  F3 = f3z^T z + f3d^T d + f3c8^T rhsc8        (PE, f32r/bf16; holds c_o*q0,
                                                pconst, angular dist, log-alpha)
  in2 = Gd*(x^2+y^2) + F3                      (ACT squares + DVE add/STT)
  S   = sum_k exp(u'_k x + v'_k)               (8 ACT exps -> bf16 slabs,
                                                DVE/GPSIMD add tree)
  wgt = exp(in2) * S                           (ACT exp + DVE mul, bf16)
  T  += That^T wgt                             (PE, bf16)

ACT is the bottleneck engine (9 exps + 2 squares per (m-tile, query) are
algorithmically required); emission order keeps its queue streaming:
Sqx, E0, Sqy0, E1, Sqy1, E2, Sqy2, E3, E4, Sqy3, E5..E7, EB per tile.
"""

import numpy as np
import ml_dtypes

B, M, N, S, K = 2048, 4096, 64, 64, 8
EPS = 1e-3
NCORES = 8
ML = M // NCORES          # 512 codebook entries per core
MT = ML // 128            # 4 m-tiles per core
NQ = 4                    # b-quarters (PSUM-sized chunks of 512)
BQ = B // NQ              # 512
f32 = np.float32
bf16 = ml_dtypes.bfloat16

_CACHE = {}


def _bf16_rt(x):
    u = np.ascontiguousarray(x, np.float32).view(np.uint32)
    r = (u + 0x7FFF + ((u >> 16) & 1)) & 0xFFFF0000
    return r.view(np.float32).astype(np.float64)


def _prep(z_re, z_im, d_re, d_im, zj_re, zj_im, dj_re, dj_im,
          That_re, That_im, alpha, sig_par, sig_perp):
    """Host-side packing: fp64 exact, cast to fp32/bf16 at the end."""
    x64 = lambda a: np.asarray(a, np.float64)
    zr, zi, dr, di = map(x64, (z_re, z_im, d_re, d_im))
    zjr, zji, djr, dji = map(x64, (zj_re, zj_im, dj_re, dj_im))

    tgl, wgl = np.polynomial.legendre.leggauss(K)
    t = (0.5 * (tgl + 1.0)).astype(f32).astype(np.float64)
    wq = (0.5 * wgl).astype(f32).astype(np.float64)

    dd2 = (djr**2 + dji**2).sum(-1)                          # [M]
    c_re = (djr * zjr + dji * zji).sum(-1)
    c_im = (djr * zji - dji * zjr).sum(-1)
    sp2 = x64(sig_par)**2 + EPS
    so2 = x64(sig_perp)**2 + EPS
    G = -0.5 / sp2
    c_o = -0.5 / so2
    Gd = G - c_o
    umid = -G * dd2
    lnal = np.log(np.maximum(x64(alpha), 1e-38))
    nzj = (zjr**2 + zji**2).sum(-1)
    nz = (zr**2 + zi**2).sum(-1)                             # [B]
    nd = (dr**2 + di**2).sum(-1)

    u = np.stack([-2.0 * G * t[k] * dd2 for k in range(K)])  # [K,M]
    up = u - umid[None, :]
    vp = np.stack([np.log(wq[k]) + G * (t[k] * dd2)**2 - up[k] * c_re
                   for k in range(K)])

    djx = np.concatenate([djr.T, dji.T], 0)                  # [128, M]
    djy = np.concatenate([-dji.T, djr.T], 0)
    f3z = ((-2.0 * c_o) * np.concatenate([zjr.T, zji.T], 0)
           + (-2.0 * Gd * c_re + umid) * djx
           + (-2.0 * Gd * c_im) * djy)
    f3d = (-2.0 * c_o) * np.concatenate([djr.T, dji.T], 0)
    const0 = (c_o * (nzj + dd2) + Gd * (c_re**2 + c_im**2)
              + lnal - umid * c_re)

    # f3c8/rhsc8: rank-8 bf16-exact update for co*(nz+nd) + pconst
    co_h = _bf16_rt(c_o)
    co_l = c_o - co_h
    pc_h = _bf16_rt(const0)
    pc_l = const0 - pc_h
    nz_h = _bf16_rt(nz)
    nz_l = nz - nz_h
    nd_h = _bf16_rt(nd)
    nd_l = nd - nd_h
    ones = np.ones_like(nz)
    f3c8 = np.stack([co_h, co_h, co_l, co_h, co_h, co_l, pc_h, pc_l])
    rhsc8 = np.stack([nz_h, nz_l, nz_h, nd_h, nd_l, nd_h, ones, ones])
    that2 = np.concatenate([x64(That_re), x64(That_im)], 1)  # [M, 128]

    # pk: per m-tile block [djx | djy | f3z], each 128 cols (f32r path)
    nt = M // 128
    pk = np.empty((128, nt * 384), np.float64)
    for j in range(nt):
        ms = slice(j * 128, (j + 1) * 128)
        pk[:, j * 384 + 0:j * 384 + 128] = djx[:, ms]
        pk[:, j * 384 + 128:j * 384 + 256] = djy[:, ms]
        pk[:, j * 384 + 256:j * 384 + 384] = f3z[:, ms]

    pGd = Gd.reshape(nt, 128).T                              # [128, nt]
    pu = up.reshape(K, nt, 128).transpose(2, 0, 1).reshape(128, K * nt)
    pv = vp.reshape(K, nt, 128).transpose(2, 0, 1).reshape(128, K * nt)

    c = lambda a: np.ascontiguousarray(a, dtype=f32)
    cb = lambda a: np.ascontiguousarray(np.asarray(a, np.float32), dtype=bf16)
    return dict(pk=c(pk), f3c8=cb(f3c8), rhsc8=cb(rhsc8), that2=cb(that2),
                f3db=cb(f3d), pGd=c(pGd), pu=c(pu), pv=c(pv),
                zst=c(np.concatenate([zr.T, zi.T], 0)),
                dstb=cb(np.concatenate([dr.T, di.T], 0)))


def _core_slices(p, cid):
    """Per-core in_map from the full packed arrays (m-sharded)."""
    jt = slice(cid * MT * 384, (cid + 1) * MT * 384)         # pk cols
    ms = slice(cid * ML, (cid + 1) * ML)
    jc = slice(cid * MT, (cid + 1) * MT)
    kc = np.concatenate([np.arange(k * (M // 128) + cid * MT,
                                   k * (M // 128) + (cid + 1) * MT)
                         for k in range(K)])
    cc = np.ascontiguousarray
    return {"pk": cc(p["pk"][:, jt]), "f3c8": cc(p["f3c8"][:, ms]),
            "that2": cc(p["that2"][ms, :]), "pGd": cc(p["pGd"][:, jc]),
            "f3db": cc(p["f3db"][:, ms]),
            "pu": cc(p["pu"][:, kc]), "pv": cc(p["pv"][:, kc]),
            "zst": p["zst"], "dstb": p["dstb"], "rhsc8": p["rhsc8"]}


def _device_maps(maps):
    dev_maps = []
    for m in maps:
        dm = {k: m[k] for k in ("pk", "f3c8", "that2", "zst", "dstb",
                                "rhsc8", "f3db")}
        dm["params"] = np.ascontiguousarray(np.concatenate(
            [m["pGd"], m["pu"], m["pv"]], axis=1))
        dev_maps.append(dm)
    return dev_maps


def _emulate_core(m):
    """Numpy emulation of one core's device program."""
    zst = np.asarray(m["zst"], f32)
    dstb = np.asarray(m["dstb"], np.float32)
    rhsc8 = np.asarray(m["rhsc8"], np.float32)
    that2 = np.asarray(m["that2"], np.float32)
    f3db = np.asarray(m["f3db"], np.float32)
    t_acc = np.zeros((128, 2048), f32)
    for j in range(MT):
        pkj = m["pk"][:, j * 384:(j + 1) * 384]
        djx_t, djy_t, f3z_t = pkj[:, 0:128], pkj[:, 128:256], pkj[:, 256:384]
        f3d_t = f3db[:, j * 128:(j + 1) * 128]
        x = (djx_t.T @ zst).astype(f32)
        y = (djy_t.T @ zst).astype(f32)
        F3 = (f3z_t.T @ zst + f3d_t.T @ dstb
              + m["f3c8"][:, j * 128:(j + 1) * 128].astype(np.float32).T
              @ rhsc8).astype(f32)
        s = (x * x + y * y).astype(f32)
        in2 = (s * m["pGd"][:, j:j + 1] + F3).astype(f32)
        Ssum = np.zeros_like(x)
        for k in range(K):
            col = k * MT + j
            arg = (x * m["pu"][:, col:col + 1] + m["pv"][:, col:col + 1]).astype(f32)
            Ssum = (Ssum + np.exp(arg, dtype=f32)).astype(f32)
        wgt = (np.exp(in2, dtype=f32) * Ssum).astype(f32)
        that_t = that2[j * 128:(j + 1) * 128, :]
        t_acc += (that_t.T @ wgt).astype(f32)
    return t_acc


def _build_bass():
    import concourse.bacc as bacc
    import concourse.mybir as mybir
    from concourse import tile

    dt = mybir.dt.float32
    f32r = mybir.dt.float32r
    bfdt = mybir.dt.bfloat16
    AF = mybir.ActivationFunctionType
    AO = mybir.AluOpType
    nc = bacc.Bacc("TRN2", target_bir_lowering=False, debug=False)

    dram = {}
    for name, shape, dty in [("zst", [128, B], f32r),
                             ("dstb", [128, B], bfdt),
                             ("rhsc8", [8, B], bfdt),
                             ("pk", [128, MT * 384], f32r),
                             ("f3c8", [8, ML], bfdt),
                             ("f3db", [128, ML], bfdt),
                             ("that2", [ML, 128], bfdt),
                             ("params", [128, MT + 2 * K * MT], dt)]:
        dram[name] = nc.dram_tensor(name, shape, dty, kind="ExternalInput")
    tout = nc.dram_tensor("tout", [128, B], dt, kind="ExternalOutput")

    HB = B // 2
    with tile.TileContext(nc) as tc:
        with tc.tile_pool(name="const", bufs=1) as cpool:
            params = cpool.tile([128, MT + 2 * K * MT], dt)
            pGd = params[:, 0:MT]
            pu = params[:, MT:MT + K * MT]
            pv = params[:, MT + K * MT:MT + 2 * K * MT]
            rhsc8 = cpool.tile([8, B], bfdt)
            that_all = cpool.tile([128, MT * 128], bfdt)
            f3c8_all = cpool.tile([8, ML], bfdt)
            f3db_all = cpool.tile([128, ML], bfdt)
            warm = cpool.tile([128, 8], dt)
            zqs = [cpool.tile([128, BQ], f32r, name=f"zq{i}") for i in range(NQ)]
            dqs = cpool.tile([128, B], bfdt)

            with (
                tc.tile_pool(name="lhs", bufs=2) as lpool,
                tc.tile_pool(name="work", bufs=1) as wpool,
                tc.tile_pool(name="eslab", bufs=2) as epool,
                tc.tile_pool(name="wgtp", bufs=1) as gpool,
            ):
                # DMA split across queues; critical path (zq, pk) on sync
                pk0x = lpool.tile([128, 128], f32r, tag="pkx")
                pk0 = lpool.tile([128, 384], f32r, tag="pk")
                nc.sync.dma_start(pk0x[:, :], dram["pk"][:, 0:128])
                nc.sync.dma_start(zqs[0][:, :], dram["zst"][:, 0:BQ])
                # warm the exp table as soon as the first DMA lands
                nc.scalar.activation(warm[:, :], pk0x[:, 0:8], AF.Exp)
                nc.sync.dma_start(zqs[1][:, :], dram["zst"][:, BQ:2 * BQ])
                nc.scalar.dma_start(zqs[2][:, :], dram["zst"][:, 2 * BQ:3 * BQ])
                nc.scalar.dma_start(zqs[3][:, :], dram["zst"][:, 3 * BQ:4 * BQ])
                nc.sync.dma_start(pk0[:, :], dram["pk"][:, 0:384])
                nc.gpsimd.dma_start(params[:, :], dram["params"][:, :])
                nc.gpsimd.dma_start(dqs[:, :], dram["dstb"][:, :])
                nc.gpsimd.dma_start(rhsc8[:, :], dram["rhsc8"][:, :])
                nc.gpsimd.dma_start(f3c8_all[:, :], dram["f3c8"][:, :])
                nc.gpsimd.dma_start(f3db_all[:, :], dram["f3db"][:, :])
                nc.gpsimd.dma_start(
                    that_all[:, :].rearrange("p (j c) -> p j c", j=MT),
                    dram["that2"][:, :].rearrange("(j p) c -> p j c", p=128))
                wgts = []
                with (
                    tc.tile_pool(name="xps", bufs=1, space="PSUM") as xpool,
                    tc.tile_pool(name="yf", bufs=2, space="PSUM") as qpool,
                    tc.tile_pool(name="tp0", bufs=1, space="PSUM") as tpool0,
                ):
                    tph0 = tpool0.tile([128, HB], dt, tag="tph0")
                    for j in range(MT):
                        if j == 0:
                            pk_t, pkx_t = pk0, pk0x
                        else:
                            pk_t = lpool.tile([128, 384], f32r, tag="pk")
                            pkx_t = lpool.tile([128, 128], f32r, tag="pkx")
                            nc.sync.dma_start(
                                pkx_t[:, :],
                                dram["pk"][:, j * 384:j * 384 + 128])
                            nc.sync.dma_start(
                                pk_t[:, :],
                                dram["pk"][:, j * 384:(j + 1) * 384])
                        djx_t = pkx_t[:, :]
                        djy_t = pk_t[:, 128:256]
                        f3z_t = pk_t[:, 256:384]
                        f3c8_t = f3c8_all[:, j * 128:(j + 1) * 128]
                        f3d_t = f3db_all[:, j * 128:(j + 1) * 128]

                        xx = wpool.tile([128, B], dt, tag="xx")
                        yy = wpool.tile([128, B], dt, tag="yy")
                        ss = wpool.tile([128, B], dt, tag="ss")
                        in2 = wpool.tile([128, B], dt, tag="in2")
                        EB = wpool.tile([128, B], bfdt, tag="EB")
                        slabs = [epool.tile([128, 2 * B], bfdt, tag=f"esl{p}",
                                            name=f"esl{p}_{j}")
                                 for p in range(4)]
                        sa = wpool.tile([128, B], bfdt, tag="sa")
                        sb = wpool.tile([128, B], bfdt, tag="sb")
                        sc = wpool.tile([128, B], bfdt, tag="sc")
                        sd = wpool.tile([128, B], bfdt, tag="sd")
                        wgt = gpool.tile([128, B], bfdt, tag=f"wgt{j}")

                        x_ps = xpool.tile([128, B], dt, tag="x")
                        for q in range(NQ):
                            nc.tensor.matmul(x_ps[:, q * BQ:(q + 1) * BQ],
                                             djx_t, zqs[q][:, :],
                                             start=True, stop=True)
                        if j > 0:
                            that_p = that_all[:, (j - 1) * 128:j * 128]
                            for q2 in range(2):
                                qs2 = slice(q2 * BQ, (q2 + 1) * BQ)
                                nc.tensor.matmul(tph0[:, qs2], that_p,
                                                 wgts[j - 1][:, qs2],
                                                 start=(j - 1 == 0),
                                                 stop=False)

                        def E(k):
                            col = k * MT + j
                            nc.scalar.activation(
                                slabs[k // 2][:, (k % 2) * B:(k % 2 + 1) * B],
                                x_ps[:, :], AF.Exp,
                                bias=pv[:, col:col + 1],
                                scale=pu[:, col:col + 1])

                        def YFpe(q):
                            """PE part: y matmul, then F3 accumulation."""
                            qs = slice(q * BQ, (q + 1) * BQ)
                            yf = qpool.tile([128, BQ], dt, tag="yf",
                                            name=f"yf{j}_{q}")
                            nc.tensor.matmul(yf[:, :], djy_t, zqs[q][:, :],
                                             start=True, stop=True)
                            return yf, qs

                        def SQY(yf, qs):
                            nc.scalar.activation(yy[:, qs], yf[:, :],
                                                 AF.Square)

                        def F3pe(yf, qs, q):
                            nc.tensor.matmul(yf[:, :], f3z_t, zqs[q][:, :],
                                             start=True, stop=False)
                            nc.tensor.matmul(yf[:, :], f3d_t,
                                             dqs[:, qs],
                                             start=False, stop=False)
                            nc.tensor.matmul(yf[:, :], f3c8_t, rhsc8[:, qs],
                                             start=False, stop=True)

                        def BASE(yf, qs):
                            nc.vector.tensor_add(ss[:, qs], xx[:, qs],
                                                 yy[:, qs])
                            nc.vector.scalar_tensor_tensor(
                                in2[:, qs], ss[:, qs], pGd[:, j:j + 1],
                                yf[:, :], AO.mult, AO.add)

                        SL = lambda p, i: slabs[p][:, i * B:(i + 1) * B]

                        # ACT queue order (strict FIFO): Sqx, E0, Sqy0,
                        # E1, Sqy1, E2, Sqy2, E3, Sqy3, E4..E7, EB0, EB1.
                        # All Sqy early so the in2 chain completes during
                        # E4..E7; EB0+EB1 after E7 cover the next tile's
                        # x-matmul chain (x_ps WAR hazard) without stalling.
                        nc.scalar.activation(xx[:, :], x_ps[:, :], AF.Square)
                        yf0, qs0 = YFpe(0)
                        E(0)
                        SQY(yf0, qs0)
                        yf1, qs1 = YFpe(1)
                        F3pe(yf0, qs0, 0)
                        E(1)
                        SQY(yf1, qs1)
                        BASE(yf0, qs0)
                        F3pe(yf1, qs1, 1)
                        E(2)
                        yf2, qs2 = YFpe(2)
                        SQY(yf2, qs2)
                        BASE(yf1, qs1)
                        F3pe(yf2, qs2, 2)
                        yf3, qs3 = YFpe(3)
                        E(3)
                        SQY(yf3, qs3)
                        BASE(yf2, qs2)
                        F3pe(yf3, qs3, 3)
                        E(4)
                        E(5)
                        BASE(yf3, qs3)
                        nc.vector.tensor_add(sa[:, :], SL(0, 0), SL(0, 1))
                        nc.gpsimd.tensor_add(sc[:, :], SL(2, 0), SL(2, 1))
                        E(6)
                        E(7)
                        nc.vector.tensor_add(sb[:, :], SL(1, 0), SL(1, 1))
                        nc.gpsimd.tensor_add(sd[:, :], SL(3, 0), SL(3, 1))
                        nc.scalar.activation(EB[:, 0:3 * BQ],
                                             in2[:, 0:3 * BQ], AF.Exp)
                        nc.scalar.activation(EB[:, 3 * BQ:B],
                                             in2[:, 3 * BQ:B], AF.Exp)
                        nc.vector.tensor_add(sa[:, :], sa[:, :], sb[:, :])
                        nc.vector.tensor_add(sc[:, :], sc[:, :], sd[:, :])
                        nc.vector.tensor_add(sa[:, :], sa[:, :], sc[:, :])
                        nc.vector.tensor_mul(wgt[:, :], EB[:, :], sa[:, :])
                        wgts.append(wgt)

                    ocp = wpool.tile([128, B], dt, tag="ocp")
                    that_l = that_all[:, (MT - 1) * 128:MT * 128]
                    for q2 in range(2):
                        qs2 = slice(q2 * BQ, (q2 + 1) * BQ)
                        nc.tensor.matmul(tph0[:, qs2], that_l,
                                         wgts[MT - 1][:, qs2],
                                         start=False, stop=True)
                        nc.vector.tensor_copy(ocp[:, qs2], tph0[:, qs2])
                        nc.sync.dma_start(tout[:, qs2], ocp[:, qs2])

                with tc.tile_pool(name="tp1", bufs=1, space="PSUM") as tpool1:
                    tph1 = tpool1.tile([128, HB], dt, tag="tph1")
                    for q2 in range(2):
                        qg = 2 + q2
                        qs = slice(qg * BQ, (qg + 1) * BQ)
                        qs2 = slice(q2 * BQ, (q2 + 1) * BQ)
                        for j in range(MT):
                            that_j = that_all[:, j * 128:(j + 1) * 128]
                            nc.tensor.matmul(tph1[:, qs2], that_j,
                                             wgts[j][:, qs],
                                             start=(j == 0),
                                             stop=(j == MT - 1))
                        nc.vector.tensor_copy(ocp[:, qs], tph1[:, qs2])
                        nc.sync.dma_start(tout[:, qs], ocp[:, qs])

    nc.compile()
    return nc


def kernel(z_re, z_im, d_re, d_im, zj_re, zj_im, dj_re, dj_im,
           That_re, That_im, alpha, sig_par, sig_perp, _emulate=False):
    p = _prep(z_re, z_im, d_re, d_im, zj_re, zj_im, dj_re, dj_im,
              That_re, That_im, alpha, sig_par, sig_perp)
    maps = [_core_slices(p, c) for c in range(NCORES)]

    if _emulate:
        outs = [_emulate_core(m) for m in maps]
    else:
        from concourse.bass_utils import run_bass_kernel_spmd
        if "nc" not in _CACHE:
            _CACHE["nc"] = _build_bass()
        dev_maps = _device_maps(maps)
        res = run_bass_kernel_spmd(_CACHE["nc"], dev_maps,
                                   core_ids=list(range(NCORES)))
        outs = [res.results[c]["tout"] for c in range(NCORES)]

    full = np.zeros((128, B), np.float64)
    for o in outs:
        full += o.astype(np.float64)
    full = full.astype(f32).T                   # [B, 128]
    return (full[:, :S] + 1j * full[:, S:]).astype(np.complex64)


# revision 7
# speedup vs baseline: 1.1066x; 1.0663x over previous
"""CPSF fused codebook kernel for Trainium2 (8 NeuronCores, codebook-parallel).

Sharding: M (codebook, 4096) split 8 ways -> 512 entries/core; every core sees
all B=2048 queries. Host sums the 8 partial [B,S] outputs.

Math (per m-tile j of 128 codebook entries):
  x = djx^T z, y = djy^T z                      (PE, f32r)
  F3 = f3z^T z + f3d^T d + f3c8^T rhsc8        (PE, f32r/bf16; holds c_o*q0,
                                                pconst, angular dist, log-alpha)
  in2 = Gd*(x^2+y^2) + F3                      (ACT squares + DVE add/STT)
  S   = sum_k exp(u'_k x + v'_k)               (8 ACT exps -> bf16 slabs,
                                                DVE/GPSIMD add tree)
  wgt = exp(in2) * S                           (ACT exp + DVE mul, bf16)
  T  += That^T wgt                             (PE, bf16)

ACT is the bottleneck engine (9 exps + 2 squares per (m-tile, query) are
algorithmically required); emission order keeps its queue streaming:
Sqx, E0, Sqy0, E1, Sqy1, E2, Sqy2, E3, E4, Sqy3, E5..E7, EB per tile.
"""

import numpy as np
import ml_dtypes

B, M, N, S, K = 2048, 4096, 64, 64, 8
EPS = 1e-3
NCORES = 8
ML = M // NCORES          # 512 codebook entries per core
MT = ML // 128            # 4 m-tiles per core
NQ = 4                    # b-quarters (PSUM-sized chunks of 512)
BQ = B // NQ              # 512
f32 = np.float32
bf16 = ml_dtypes.bfloat16

_CACHE = {}


def _bf16_rt(x):
    u = np.ascontiguousarray(x, np.float32).view(np.uint32)
    r = (u + 0x7FFF + ((u >> 16) & 1)) & 0xFFFF0000
    return r.view(np.float32).astype(np.float64)


def _prep(z_re, z_im, d_re, d_im, zj_re, zj_im, dj_re, dj_im,
          That_re, That_im, alpha, sig_par, sig_perp):
    """Host-side packing: fp64 exact, cast to fp32/bf16 at the end."""
    x64 = lambda a: np.asarray(a, np.float64)
    zr, zi, dr, di = map(x64, (z_re, z_im, d_re, d_im))
    zjr, zji, djr, dji = map(x64, (zj_re, zj_im, dj_re, dj_im))

    tgl, wgl = np.polynomial.legendre.leggauss(K)
    t = (0.5 * (tgl + 1.0)).astype(f32).astype(np.float64)
    wq = (0.5 * wgl).astype(f32).astype(np.float64)

    dd2 = (djr**2 + dji**2).sum(-1)                          # [M]
    c_re = (djr * zjr + dji * zji).sum(-1)
    c_im = (djr * zji - dji * zjr).sum(-1)
    sp2 = x64(sig_par)**2 + EPS
    so2 = x64(sig_perp)**2 + EPS
    G = -0.5 / sp2
    c_o = -0.5 / so2
    Gd = G - c_o
    umid = -G * dd2
    lnal = np.log(np.maximum(x64(alpha), 1e-38))
    nzj = (zjr**2 + zji**2).sum(-1)
    nz = (zr**2 + zi**2).sum(-1)                             # [B]
    nd = (dr**2 + di**2).sum(-1)

    u = np.stack([-2.0 * G * t[k] * dd2 for k in range(K)])  # [K,M]
    up = u - umid[None, :]
    vp = np.stack([np.log(wq[k]) + G * (t[k] * dd2)**2 - up[k] * c_re
                   for k in range(K)])

    djx = np.concatenate([djr.T, dji.T], 0)                  # [128, M]
    djy = np.concatenate([-dji.T, djr.T], 0)
    f3z = ((-2.0 * c_o) * np.concatenate([zjr.T, zji.T], 0)
           + (-2.0 * Gd * c_re + umid) * djx
           + (-2.0 * Gd * c_im) * djy)
    f3d = (-2.0 * c_o) * np.concatenate([djr.T, dji.T], 0)
    const0 = (c_o * (nzj + dd2) + Gd * (c_re**2 + c_im**2)
              + lnal - umid * c_re)

    # f3c8/rhsc8: rank-8 bf16-exact update for co*(nz+nd) + pconst
    co_h = _bf16_rt(c_o)
    co_l = c_o - co_h
    pc_h = _bf16_rt(const0)
    pc_l = const0 - pc_h
    nz_h = _bf16_rt(nz)
    nz_l = nz - nz_h
    nd_h = _bf16_rt(nd)
    nd_l = nd - nd_h
    ones = np.ones_like(nz)
    f3c8 = np.stack([co_h, co_h, co_l, co_h, co_h, co_l, pc_h, pc_l])
    rhsc8 = np.stack([nz_h, nz_l, nz_h, nd_h, nd_l, nd_h, ones, ones])
    that2 = np.concatenate([x64(That_re), x64(That_im)], 1)  # [M, 128]

    # pk: per m-tile block [djx | djy | f3z], each 128 cols (f32r path)
    nt = M // 128
    pk = np.empty((128, nt * 384), np.float64)
    for j in range(nt):
        ms = slice(j * 128, (j + 1) * 128)
        pk[:, j * 384 + 0:j * 384 + 128] = djx[:, ms]
        pk[:, j * 384 + 128:j * 384 + 256] = djy[:, ms]
        pk[:, j * 384 + 256:j * 384 + 384] = f3z[:, ms]

    pGd = Gd.reshape(nt, 128).T                              # [128, nt]
    pu = up.reshape(K, nt, 128).transpose(2, 0, 1).reshape(128, K * nt)
    pv = vp.reshape(K, nt, 128).transpose(2, 0, 1).reshape(128, K * nt)

    c = lambda a: np.ascontiguousarray(a, dtype=f32)
    cb = lambda a: np.ascontiguousarray(np.asarray(a, np.float32), dtype=bf16)
    return dict(pk=c(pk), f3c8=cb(f3c8), rhsc8=cb(rhsc8), that2=cb(that2),
                f3db=cb(f3d), pGd=c(pGd), pu=c(pu), pv=c(pv),
                zst=c(np.concatenate([zr.T, zi.T], 0)),
                dstb=cb(np.concatenate([dr.T, di.T], 0)))


def _core_slices(p, cid):
    """Per-core in_map from the full packed arrays (m-sharded)."""
    jt = slice(cid * MT * 384, (cid + 1) * MT * 384)         # pk cols
    ms = slice(cid * ML, (cid + 1) * ML)
    jc = slice(cid * MT, (cid + 1) * MT)
    kc = np.concatenate([np.arange(k * (M // 128) + cid * MT,
                                   k * (M // 128) + (cid + 1) * MT)
                         for k in range(K)])
    cc = np.ascontiguousarray
    return {"pk": cc(p["pk"][:, jt]), "f3c8": cc(p["f3c8"][:, ms]),
            "that2": cc(p["that2"][ms, :]), "pGd": cc(p["pGd"][:, jc]),
            "f3db": cc(p["f3db"][:, ms]),
            "pu": cc(p["pu"][:, kc]), "pv": cc(p["pv"][:, kc]),
            "zst": p["zst"], "dstb": p["dstb"], "rhsc8": p["rhsc8"]}


def _device_maps(maps):
    dev_maps = []
    for m in maps:
        dm = {k: m[k] for k in ("pk", "f3c8", "that2", "zst", "dstb",
                                "rhsc8", "f3db")}
        dm["params"] = np.ascontiguousarray(np.concatenate(
            [m["pGd"], m["pu"], m["pv"]], axis=1))
        dev_maps.append(dm)
    return dev_maps


def _emulate_core(m):
    """Numpy emulation of one core's device program."""
    zst = np.asarray(m["zst"], f32)
    dstb = np.asarray(m["dstb"], np.float32)
    rhsc8 = np.asarray(m["rhsc8"], np.float32)
    that2 = np.asarray(m["that2"], np.float32)
    f3db = np.asarray(m["f3db"], np.float32)
    t_acc = np.zeros((128, 2048), f32)
    for j in range(MT):
        pkj = m["pk"][:, j * 384:(j + 1) * 384]
        djx_t, djy_t, f3z_t = pkj[:, 0:128], pkj[:, 128:256], pkj[:, 256:384]
        f3d_t = f3db[:, j * 128:(j + 1) * 128]
        x = (djx_t.T @ zst).astype(f32)
        y = (djy_t.T @ zst).astype(f32)
        F3 = (f3z_t.T @ zst + f3d_t.T @ dstb
              + m["f3c8"][:, j * 128:(j + 1) * 128].astype(np.float32).T
              @ rhsc8).astype(f32)
        s = (x * x + y * y).astype(f32)
        in2 = (s * m["pGd"][:, j:j + 1] + F3).astype(f32)
        Ssum = np.zeros_like(x)
        for k in range(K):
            col = k * MT + j
            arg = (x * m["pu"][:, col:col + 1] + m["pv"][:, col:col + 1]).astype(f32)
            Ssum = (Ssum + np.exp(arg, dtype=f32)).astype(f32)
        wgt = (np.exp(in2, dtype=f32) * Ssum).astype(f32)
        that_t = that2[j * 128:(j + 1) * 128, :]
        t_acc += (that_t.T @ wgt).astype(f32)
    return t_acc


def _build_bass():
    import concourse.bacc as bacc
    import concourse.mybir as mybir
    from concourse import tile

    dt = mybir.dt.float32
    f32r = mybir.dt.float32r
    bfdt = mybir.dt.bfloat16
    AF = mybir.ActivationFunctionType
    AO = mybir.AluOpType
    nc = bacc.Bacc("TRN2", target_bir_lowering=False, debug=False)

    dram = {}
    for name, shape, dty in [("zst", [128, B], f32r),
                             ("dstb", [128, B], bfdt),
                             ("rhsc8", [8, B], bfdt),
                             ("pk", [128, MT * 384], f32r),
                             ("f3c8", [8, ML], bfdt),
                             ("f3db", [128, ML], bfdt),
                             ("that2", [ML, 128], bfdt),
                             ("params", [128, MT + 2 * K * MT], dt)]:
        dram[name] = nc.dram_tensor(name, shape, dty, kind="ExternalInput")
    tout = nc.dram_tensor("tout", [128, B], dt, kind="ExternalOutput")

    HB = B // 2
    with tile.TileContext(nc) as tc:
        with tc.tile_pool(name="const", bufs=1) as cpool:
            params = cpool.tile([128, MT + 2 * K * MT], dt)
            pGd = params[:, 0:MT]
            pu = params[:, MT:MT + K * MT]
            pv = params[:, MT + K * MT:MT + 2 * K * MT]
            rhsc8 = cpool.tile([8, B], bfdt)
            that_all = cpool.tile([128, MT * 128], bfdt)
            f3c8_all = cpool.tile([8, ML], bfdt)
            f3db_all = cpool.tile([128, ML], bfdt)
            warm = cpool.tile([128, 8], dt)
            zqs = [cpool.tile([128, BQ], f32r, name=f"zq{i}") for i in range(NQ)]
            dqs = cpool.tile([128, B], bfdt)

            with (
                tc.tile_pool(name="lhs", bufs=2) as lpool,
                tc.tile_pool(name="work", bufs=1) as wpool,
                tc.tile_pool(name="eslab", bufs=2) as epool,
                tc.tile_pool(name="wgtp", bufs=1) as gpool,
            ):
                # Critical-path DMAs (zq, pk, params) on sync, in
                # priority order.  Bulk DMAs go on the gpsimd queue but are
                # gated behind zq3's arrival by a tiny gpsimd copy, so they
                # don't steal HBM bandwidth from the startup critical path.
                pk0x = lpool.tile([128, 128], f32r, tag="pkx")
                pk0 = lpool.tile([128, 384], f32r, tag="pk")
                nc.sync.dma_start(pk0x[:, :], dram["pk"][:, 0:128])
                nc.sync.dma_start(zqs[0][:, :], dram["zst"][:, 0:BQ])
                # warm the exp table as soon as the first DMA lands
                nc.scalar.activation(warm[:, :], pk0x[:, 0:8], AF.Exp)
                nc.sync.dma_start(zqs[1][:, :], dram["zst"][:, BQ:2 * BQ])
                nc.sync.dma_start(zqs[2][:, :], dram["zst"][:, 2 * BQ:3 * BQ])
                nc.sync.dma_start(zqs[3][:, :], dram["zst"][:, 3 * BQ:4 * BQ])
                nc.sync.dma_start(pk0[:, :], dram["pk"][:, 0:384])
                nc.sync.dma_start(params[:, :], dram["params"][:, :])
                nc.gpsimd.tensor_copy(warm[:, 0:8], zqs[3][:, 0:8])
                nc.gpsimd.dma_start(f3db_all[:, :], dram["f3db"][:, :])
                nc.gpsimd.dma_start(dqs[:, 0:BQ], dram["dstb"][:, 0:BQ])
                nc.gpsimd.dma_start(rhsc8[:, :], dram["rhsc8"][:, :])
                nc.gpsimd.dma_start(f3c8_all[:, :], dram["f3c8"][:, :])
                nc.gpsimd.dma_start(dqs[:, BQ:B], dram["dstb"][:, BQ:B])
                nc.gpsimd.dma_start(
                    that_all[:, :].rearrange("p (j c) -> p j c", j=MT),
                    dram["that2"][:, :].rearrange("(j p) c -> p j c", p=128))
                wgts = []
                with (
                    tc.tile_pool(name="xps", bufs=1, space="PSUM") as xpool,
                    tc.tile_pool(name="yf", bufs=2, space="PSUM") as qpool,
                    tc.tile_pool(name="tp0", bufs=1, space="PSUM") as tpool0,
                ):
                    tph0 = tpool0.tile([128, HB], dt, tag="tph0")
                    for j in range(MT):
                        if j == 0:
                            pk_t, pkx_t = pk0, pk0x
                        else:
                            pk_t = lpool.tile([128, 384], f32r, tag="pk")
                            pkx_t = lpool.tile([128, 128], f32r, tag="pkx")
                            nc.sync.dma_start(
                                pkx_t[:, :],
                                dram["pk"][:, j * 384:j * 384 + 128])
                            nc.sync.dma_start(
                                pk_t[:, :],
                                dram["pk"][:, j * 384:(j + 1) * 384])
                        djx_t = pkx_t[:, :]
                        djy_t = pk_t[:, 128:256]
                        f3z_t = pk_t[:, 256:384]
                        f3c8_t = f3c8_all[:, j * 128:(j + 1) * 128]
                        f3d_t = f3db_all[:, j * 128:(j + 1) * 128]

                        xx = wpool.tile([128, B], dt, tag="xx")
                        yy = wpool.tile([128, B], dt, tag="yy")
                        ss = wpool.tile([128, B], dt, tag="ss")
                        in2 = wpool.tile([128, B], dt, tag="in2")
                        EB = wpool.tile([128, B], bfdt, tag="EB")
                        slabs = [epool.tile([128, 2 * B], bfdt, tag=f"esl{p}",
                                            name=f"esl{p}_{j}")
                                 for p in range(4)]
                        sa = wpool.tile([128, B], bfdt, tag="sa")
                        sb = wpool.tile([128, B], bfdt, tag="sb")
                        sc = wpool.tile([128, B], bfdt, tag="sc")
                        sd = wpool.tile([128, B], bfdt, tag="sd")
                        wgt = gpool.tile([128, B], bfdt, tag=f"wgt{j}")

                        x_ps = xpool.tile([128, B], dt, tag="x")
                        yf_early = []
                        for q in range(2):
                            yf = qpool.tile([128, BQ], dt, tag="yf",
                                            name=f"yf{j}_{q}")
                            nc.tensor.matmul(yf[:, :], djy_t, zqs[q][:, :],
                                             start=True, stop=True)
                            yf_early.append((yf, slice(q * BQ, (q + 1) * BQ)))
                        for q in range(NQ):
                            nc.tensor.matmul(x_ps[:, q * BQ:(q + 1) * BQ],
                                             djx_t, zqs[q][:, :],
                                             start=True, stop=True)
                        if j > 0:
                            that_p = that_all[:, (j - 1) * 128:j * 128]
                            for q2 in range(2):
                                qs2 = slice(q2 * BQ, (q2 + 1) * BQ)
                                nc.tensor.matmul(tph0[:, qs2], that_p,
                                                 wgts[j - 1][:, qs2],
                                                 start=(j - 1 == 0),
                                                 stop=False)

                        def E(k):
                            col = k * MT + j
                            nc.scalar.activation(
                                slabs[k // 2][:, (k % 2) * B:(k % 2 + 1) * B],
                                x_ps[:, :], AF.Exp,
                                bias=pv[:, col:col + 1],
                                scale=pu[:, col:col + 1])

                        def YFpe(q):
                            """PE part: y matmul (q0/q1 pre-issued)."""
                            if q < 2:
                                return yf_early[q]
                            qs = slice(q * BQ, (q + 1) * BQ)
                            yf = qpool.tile([128, BQ], dt, tag="yf",
                                            name=f"yf{j}_{q}")
                            nc.tensor.matmul(yf[:, :], djy_t, zqs[q][:, :],
                                             start=True, stop=True)
                            return yf, qs

                        def SQY(yf, qs):
                            nc.scalar.activation(yy[:, qs], yf[:, :],
                                                 AF.Square)

                        def F3pe(yf, qs, q):
                            nc.tensor.matmul(yf[:, :], f3z_t, zqs[q][:, :],
                                             start=True, stop=False)
                            nc.tensor.matmul(yf[:, :], f3d_t,
                                             dqs[:, qs],
                                             start=False, stop=False)
                            nc.tensor.matmul(yf[:, :], f3c8_t, rhsc8[:, qs],
                                             start=False, stop=True)

                        def BASE(yf, qs):
                            nc.vector.tensor_add(ss[:, qs], xx[:, qs],
                                                 yy[:, qs])
                            nc.vector.scalar_tensor_tensor(
                                in2[:, qs], ss[:, qs], pGd[:, j:j + 1],
                                yf[:, :], AO.mult, AO.add)

                        SL = lambda p, i: slabs[p][:, i * B:(i + 1) * B]

                        # ACT queue order (strict FIFO): Sqx, E0, Sqy0,
                        # E1, Sqy1, E2, Sqy2, E3, Sqy3, E4..E7, EB0, EB1.
                        # All Sqy early so the in2 chain completes during
                        # E4..E7; EB0+EB1 after E7 cover the next tile's
                        # x-matmul chain (x_ps WAR hazard) without stalling.
                        nc.scalar.activation(xx[:, :], x_ps[:, :], AF.Square)
                        yf0, qs0 = YFpe(0)
                        E(0)
                        SQY(yf0, qs0)
                        yf1, qs1 = YFpe(1)
                        F3pe(yf0, qs0, 0)
                        E(1)
                        SQY(yf1, qs1)
                        BASE(yf0, qs0)
                        F3pe(yf1, qs1, 1)
                        E(2)
                        yf2, qs2 = YFpe(2)
                        SQY(yf2, qs2)
                        BASE(yf1, qs1)
                        F3pe(yf2, qs2, 2)
                        yf3, qs3 = YFpe(3)
                        E(3)
                        SQY(yf3, qs3)
                        BASE(yf2, qs2)
                        F3pe(yf3, qs3, 3)
                        nc.vector.tensor_add(sa[:, :], SL(0, 0), SL(0, 1))
                        nc.gpsimd.tensor_add(sc[:, :], SL(1, 0), SL(1, 1))
                        E(4)
                        E(5)
                        BASE(yf3, qs3)
                        nc.gpsimd.tensor_add(sd[:, :], SL(2, 0), SL(2, 1))
                        nc.vector.tensor_add(sb[:, :], sa[:, :], sc[:, :])
                        E(6)
                        nc.vector.tensor_add(sa[:, :], sb[:, :], sd[:, :])
                        E(7)
                        nc.scalar.activation(EB[:, 0:3 * BQ],
                                             in2[:, 0:3 * BQ], AF.Exp)
                        nc.scalar.activation(EB[:, 3 * BQ:B],
                                             in2[:, 3 * BQ:B], AF.Exp)
                        nc.vector.tensor_add(sb[:, :], sa[:, :], SL(3, 0))
                        nc.vector.tensor_add(sa[:, :], sb[:, :], SL(3, 1))
                        nc.vector.tensor_mul(wgt[:, :], EB[:, :], sa[:, :])
                        wgts.append(wgt)

                    ocp = wpool.tile([128, B], dt, tag="ocp")
                    that_l = that_all[:, (MT - 1) * 128:MT * 128]
                    for q2 in range(2):
                        qs2 = slice(q2 * BQ, (q2 + 1) * BQ)
                        nc.tensor.matmul(tph0[:, qs2], that_l,
                                         wgts[MT - 1][:, qs2],
                                         start=False, stop=True)
                        nc.vector.tensor_copy(ocp[:, qs2], tph0[:, qs2])
                        nc.sync.dma_start(tout[:, qs2], ocp[:, qs2])

                with tc.tile_pool(name="tp1", bufs=1, space="PSUM") as tpool1:
                    tph1 = tpool1.tile([128, HB], dt, tag="tph1")
                    for q2 in range(2):
                        qg = 2 + q2
                        qs = slice(qg * BQ, (qg + 1) * BQ)
                        qs2 = slice(q2 * BQ, (q2 + 1) * BQ)
                        for j in range(MT):
                            that_j = that_all[:, j * 128:(j + 1) * 128]
                            nc.tensor.matmul(tph1[:, qs2], that_j,
                                             wgts[j][:, qs],
                                             start=(j == 0),
                                             stop=(j == MT - 1))
                        nc.vector.tensor_copy(ocp[:, qs], tph1[:, qs2])
                        nc.sync.dma_start(tout[:, qs], ocp[:, qs])

    nc.compile()
    return nc


def kernel(z_re, z_im, d_re, d_im, zj_re, zj_im, dj_re, dj_im,
           That_re, That_im, alpha, sig_par, sig_perp, _emulate=False):
    p = _prep(z_re, z_im, d_re, d_im, zj_re, zj_im, dj_re, dj_im,
              That_re, That_im, alpha, sig_par, sig_perp)
    maps = [_core_slices(p, c) for c in range(NCORES)]

    if _emulate:
        outs = [_emulate_core(m) for m in maps]
    else:
        from concourse.bass_utils import run_bass_kernel_spmd
        if "nc" not in _CACHE:
            _CACHE["nc"] = _build_bass()
        dev_maps = _device_maps(maps)
        res = run_bass_kernel_spmd(_CACHE["nc"], dev_maps,
                                   core_ids=list(range(NCORES)))
        outs = [res.results[c]["tout"] for c in range(NCORES)]

    full = np.zeros((128, B), np.float64)
    for o in outs:
        full += o.astype(np.float64)
    full = full.astype(f32).T                   # [B, 128]
    return (full[:, :S] + 1j * full[:, S:]).astype(np.complex64)


# revision 8
# speedup vs baseline: 1.1293x; 1.0205x over previous
"""CPSF fused codebook kernel for Trainium2 (8 NeuronCores, codebook-parallel).

Sharding: M (codebook, 4096) split 8 ways -> 512 entries/core; every core sees
all B=2048 queries. Host sums the 8 partial [B,S] outputs.

Math (per m-tile j of 128 codebook entries):
  x = djx^T z, y = djy^T z                      (PE, f32r)
  F3 = f3z^T z + f3d^T d + f3c8^T rhsc8        (PE, f32r/bf16; holds c_o*q0,
                                                pconst, angular dist, log-alpha)
  in2 = Gd*(x^2+y^2) + F3                      (ACT squares + DVE add/STT)
  S   = sum_k exp(u'_k x + v'_k)               (8 ACT exps -> bf16 slabs,
                                                DVE/GPSIMD add tree)
  wgt = exp(in2) * S                           (ACT exp + DVE mul, bf16)
  T  += That^T wgt                             (PE, bf16)

ACT is the bottleneck engine (9 exps + 2 squares per (m-tile, query) are
algorithmically required); emission order keeps its queue streaming:
Sqx, E0, Sqy0, E1, Sqy1, E2, Sqy2, E3, E4, Sqy3, E5..E7, EB per tile.
"""

import numpy as np
import ml_dtypes

B, M, N, S, K = 2048, 4096, 64, 64, 8
EPS = 1e-3
NCORES = 8
ML = M // NCORES          # 512 codebook entries per core
MT = ML // 128            # 4 m-tiles per core
NQ = 4                    # b-quarters (PSUM-sized chunks of 512)
BQ = B // NQ              # 512
f32 = np.float32
bf16 = ml_dtypes.bfloat16

_CACHE = {}


def _bf16_rt(x):
    u = np.ascontiguousarray(x, np.float32).view(np.uint32)
    r = (u + 0x7FFF + ((u >> 16) & 1)) & 0xFFFF0000
    return r.view(np.float32).astype(np.float64)


def _prep(z_re, z_im, d_re, d_im, zj_re, zj_im, dj_re, dj_im,
          That_re, That_im, alpha, sig_par, sig_perp):
    """Host-side packing: fp64 exact, cast to fp32/bf16 at the end."""
    x64 = lambda a: np.asarray(a, np.float64)
    zr, zi, dr, di = map(x64, (z_re, z_im, d_re, d_im))
    zjr, zji, djr, dji = map(x64, (zj_re, zj_im, dj_re, dj_im))

    tgl, wgl = np.polynomial.legendre.leggauss(K)
    t = (0.5 * (tgl + 1.0)).astype(f32).astype(np.float64)
    wq = (0.5 * wgl).astype(f32).astype(np.float64)

    dd2 = (djr**2 + dji**2).sum(-1)                          # [M]
    c_re = (djr * zjr + dji * zji).sum(-1)
    c_im = (djr * zji - dji * zjr).sum(-1)
    sp2 = x64(sig_par)**2 + EPS
    so2 = x64(sig_perp)**2 + EPS
    G = -0.5 / sp2
    c_o = -0.5 / so2
    Gd = G - c_o
    umid = -G * dd2
    lnal = np.log(np.maximum(x64(alpha), 1e-38))
    nzj = (zjr**2 + zji**2).sum(-1)
    nz = (zr**2 + zi**2).sum(-1)                             # [B]
    nd = (dr**2 + di**2).sum(-1)

    u = np.stack([-2.0 * G * t[k] * dd2 for k in range(K)])  # [K,M]
    up = u - umid[None, :]
    vp = np.stack([np.log(wq[k]) + G * (t[k] * dd2)**2 - up[k] * c_re
                   for k in range(K)])

    djx = np.concatenate([djr.T, dji.T], 0)                  # [128, M]
    djy = np.concatenate([-dji.T, djr.T], 0)
    f3z = ((-2.0 * c_o) * np.concatenate([zjr.T, zji.T], 0)
           + (-2.0 * Gd * c_re + umid) * djx
           + (-2.0 * Gd * c_im) * djy)
    f3d = (-2.0 * c_o) * np.concatenate([djr.T, dji.T], 0)
    const0 = (c_o * (nzj + dd2) + Gd * (c_re**2 + c_im**2)
              + lnal - umid * c_re)

    # f3c8/rhsc8: rank-8 bf16-exact update for co*(nz+nd) + pconst
    co_h = _bf16_rt(c_o)
    co_l = c_o - co_h
    pc_h = _bf16_rt(const0)
    pc_l = const0 - pc_h
    nz_h = _bf16_rt(nz)
    nz_l = nz - nz_h
    nd_h = _bf16_rt(nd)
    nd_l = nd - nd_h
    ones = np.ones_like(nz)
    f3c8 = np.stack([co_h, co_h, co_l, co_h, co_h, co_l, pc_h, pc_l])
    rhsc8 = np.stack([nz_h, nz_l, nz_h, nd_h, nd_l, nd_h, ones, ones])
    that2 = np.concatenate([x64(That_re), x64(That_im)], 1)  # [M, 128]

    # pk: per m-tile block [djx | djy | f3z], each 128 cols (f32r path)
    nt = M // 128
    pk = np.empty((128, nt * 384), np.float64)
    for j in range(nt):
        ms = slice(j * 128, (j + 1) * 128)
        pk[:, j * 384 + 0:j * 384 + 128] = djx[:, ms]
        pk[:, j * 384 + 128:j * 384 + 256] = djy[:, ms]
        pk[:, j * 384 + 256:j * 384 + 384] = f3z[:, ms]

    pGd = Gd.reshape(nt, 128).T                              # [128, nt]
    pu = up.reshape(K, nt, 128).transpose(2, 0, 1).reshape(128, K * nt)
    pv = vp.reshape(K, nt, 128).transpose(2, 0, 1).reshape(128, K * nt)

    c = lambda a: np.ascontiguousarray(a, dtype=f32)
    cb = lambda a: np.ascontiguousarray(np.asarray(a, np.float32), dtype=bf16)
    return dict(pk=c(pk), f3c8=cb(f3c8), rhsc8=cb(rhsc8), that2=cb(that2),
                f3db=cb(f3d), pGd=c(pGd), pu=c(pu), pv=c(pv),
                zst=c(np.concatenate([zr.T, zi.T], 0)),
                dstb=cb(np.concatenate([dr.T, di.T], 0)))


def _core_slices(p, cid):
    """Per-core in_map from the full packed arrays (m-sharded)."""
    jt = slice(cid * MT * 384, (cid + 1) * MT * 384)         # pk cols
    ms = slice(cid * ML, (cid + 1) * ML)
    jc = slice(cid * MT, (cid + 1) * MT)
    kc = np.concatenate([np.arange(k * (M // 128) + cid * MT,
                                   k * (M // 128) + (cid + 1) * MT)
                         for k in range(K)])
    cc = np.ascontiguousarray
    return {"pk": cc(p["pk"][:, jt]), "f3c8": cc(p["f3c8"][:, ms]),
            "that2": cc(p["that2"][ms, :]), "pGd": cc(p["pGd"][:, jc]),
            "f3db": cc(p["f3db"][:, ms]),
            "pu": cc(p["pu"][:, kc]), "pv": cc(p["pv"][:, kc]),
            "zst": p["zst"], "dstb": p["dstb"], "rhsc8": p["rhsc8"]}


def _device_maps(maps):
    dev_maps = []
    for m in maps:
        dm = {k: m[k] for k in ("pk", "f3c8", "that2", "zst", "dstb",
                                "rhsc8", "f3db")}
        dm["params"] = np.ascontiguousarray(np.concatenate(
            [m["pGd"], m["pu"], m["pv"]], axis=1))
        dev_maps.append(dm)
    return dev_maps


def _emulate_core(m):
    """Numpy emulation of one core's device program."""
    zst = np.asarray(m["zst"], f32)
    dstb = np.asarray(m["dstb"], np.float32)
    rhsc8 = np.asarray(m["rhsc8"], np.float32)
    that2 = np.asarray(m["that2"], np.float32)
    f3db = np.asarray(m["f3db"], np.float32)
    t_acc = np.zeros((128, 2048), f32)
    for j in range(MT):
        pkj = m["pk"][:, j * 384:(j + 1) * 384]
        djx_t, djy_t, f3z_t = pkj[:, 0:128], pkj[:, 128:256], pkj[:, 256:384]
        f3d_t = f3db[:, j * 128:(j + 1) * 128]
        x = (djx_t.T @ zst).astype(f32)
        y = (djy_t.T @ zst).astype(f32)
        F3 = (f3z_t.T @ zst + f3d_t.T @ dstb
              + m["f3c8"][:, j * 128:(j + 1) * 128].astype(np.float32).T
              @ rhsc8).astype(f32)
        s = (x * x + y * y).astype(f32)
        in2 = (s * m["pGd"][:, j:j + 1] + F3).astype(f32)
        Ssum = np.zeros_like(x)
        for k in range(K):
            col = k * MT + j
            arg = (x * m["pu"][:, col:col + 1] + m["pv"][:, col:col + 1]).astype(f32)
            Ssum = (Ssum + np.exp(arg, dtype=f32)).astype(f32)
        wgt = (np.exp(in2, dtype=f32) * Ssum).astype(f32)
        that_t = that2[j * 128:(j + 1) * 128, :]
        t_acc += (that_t.T @ wgt).astype(f32)
    return t_acc


def _build_bass():
    import concourse.bacc as bacc
    import concourse.mybir as mybir
    from concourse import tile
    import concourse.hw_specs as _hwsp
    # The scheduler's cost sim models f32r matmuls at 1 cyc/row; real HW
    # streams them at ~2 cyc/row. Halving the modeled PE clock makes the
    # static schedule place ACT filler ops into the real matmul shadows.
    _hwsp.TRN2Spec.PE_CYCLE = 1e9 / 1.2e9

    dt = mybir.dt.float32
    f32r = mybir.dt.float32r
    bfdt = mybir.dt.bfloat16
    AF = mybir.ActivationFunctionType
    AO = mybir.AluOpType
    nc = bacc.Bacc("TRN2", target_bir_lowering=False, debug=False)

    dram = {}
    for name, shape, dty in [("zst", [128, B], f32r),
                             ("dstb", [128, B], bfdt),
                             ("rhsc8", [8, B], bfdt),
                             ("pk", [128, MT * 384], f32r),
                             ("f3c8", [8, ML], bfdt),
                             ("f3db", [128, ML], bfdt),
                             ("that2", [ML, 128], bfdt),
                             ("params", [128, MT + 2 * K * MT], dt)]:
        dram[name] = nc.dram_tensor(name, shape, dty, kind="ExternalInput")
    tout = nc.dram_tensor("tout", [128, B], dt, kind="ExternalOutput")

    HB = B // 2
    with tile.TileContext(nc) as tc:
        with tc.tile_pool(name="const", bufs=1) as cpool:
            params = cpool.tile([128, MT + 2 * K * MT], dt)
            pGd = params[:, 0:MT]
            pu = params[:, MT:MT + K * MT]
            pv = params[:, MT + K * MT:MT + 2 * K * MT]
            rhsc8 = cpool.tile([8, B], bfdt)
            that_all = cpool.tile([128, MT * 128], bfdt)
            f3c8_all = cpool.tile([8, ML], bfdt)
            f3db_all = cpool.tile([128, ML], bfdt)
            warm = cpool.tile([128, 8], dt)
            zqs = [cpool.tile([128, BQ], f32r, name=f"zq{i}") for i in range(NQ)]
            dqs = cpool.tile([128, B], bfdt)

            with (
                tc.tile_pool(name="lhs", bufs=2) as lpool,
                tc.tile_pool(name="work", bufs=1) as wpool,
                tc.tile_pool(name="eslab", bufs=2) as epool,
                tc.tile_pool(name="wgtp", bufs=1) as gpool,
            ):
                # Critical-path DMAs (zq, pk, params) on sync, in
                # priority order.  Bulk DMAs go on the gpsimd queue but are
                # gated behind zq3's arrival by a tiny gpsimd copy, so they
                # don't steal HBM bandwidth from the startup critical path.
                pk0x = lpool.tile([128, 128], f32r, tag="pkx")
                pk0 = lpool.tile([128, 384], f32r, tag="pk")
                nc.sync.dma_start(pk0x[:, :], dram["pk"][:, 0:128])
                nc.sync.dma_start(zqs[0][:, :], dram["zst"][:, 0:BQ])
                # warm the exp table as soon as the first DMA lands
                nc.scalar.activation(warm[:, :], pk0x[:, 0:8], AF.Exp)
                nc.sync.dma_start(zqs[1][:, :], dram["zst"][:, BQ:2 * BQ])
                nc.sync.dma_start(zqs[2][:, :], dram["zst"][:, 2 * BQ:3 * BQ])
                nc.sync.dma_start(zqs[3][:, :], dram["zst"][:, 3 * BQ:4 * BQ])
                nc.sync.dma_start(pk0[:, :], dram["pk"][:, 0:384])
                nc.sync.dma_start(params[:, :], dram["params"][:, :])
                nc.gpsimd.tensor_copy(warm[:, 0:8], zqs[3][:, 0:8])
                nc.gpsimd.dma_start(f3db_all[:, :], dram["f3db"][:, :])
                nc.gpsimd.dma_start(dqs[:, 0:BQ], dram["dstb"][:, 0:BQ])
                nc.gpsimd.dma_start(rhsc8[:, :], dram["rhsc8"][:, :])
                nc.gpsimd.dma_start(f3c8_all[:, :], dram["f3c8"][:, :])
                nc.gpsimd.dma_start(dqs[:, BQ:B], dram["dstb"][:, BQ:B])
                nc.gpsimd.dma_start(
                    that_all[:, :].rearrange("p (j c) -> p j c", j=MT),
                    dram["that2"][:, :].rearrange("(j p) c -> p j c", p=128))
                wgts = []
                with (
                    tc.tile_pool(name="xps", bufs=1, space="PSUM") as xpool,
                    tc.tile_pool(name="yf", bufs=2, space="PSUM") as qpool,
                    tc.tile_pool(name="tp0", bufs=1, space="PSUM") as tpool0,
                ):
                    tph0 = tpool0.tile([128, HB], dt, tag="tph0")
                    for j in range(MT):
                        if j == 0:
                            pk_t, pkx_t = pk0, pk0x
                        else:
                            pk_t = lpool.tile([128, 384], f32r, tag="pk")
                            pkx_t = lpool.tile([128, 128], f32r, tag="pkx")
                            nc.sync.dma_start(
                                pkx_t[:, :],
                                dram["pk"][:, j * 384:j * 384 + 128])
                            nc.sync.dma_start(
                                pk_t[:, :],
                                dram["pk"][:, j * 384:(j + 1) * 384])
                        djx_t = pkx_t[:, :]
                        djy_t = pk_t[:, 128:256]
                        f3z_t = pk_t[:, 256:384]
                        f3c8_t = f3c8_all[:, j * 128:(j + 1) * 128]
                        f3d_t = f3db_all[:, j * 128:(j + 1) * 128]

                        xx = wpool.tile([128, B], dt, tag="xx")
                        yy = wpool.tile([128, B], dt, tag="yy")
                        ss = wpool.tile([128, B], dt, tag="ss")
                        in2 = wpool.tile([128, B], dt, tag="in2")
                        EB = wpool.tile([128, B], bfdt, tag="EB")
                        slabs = [epool.tile([128, 2 * B], bfdt, tag=f"esl{p}",
                                            name=f"esl{p}_{j}")
                                 for p in range(4)]
                        sa = wpool.tile([128, B], bfdt, tag="sa")
                        sb = wpool.tile([128, B], bfdt, tag="sb")
                        sc = wpool.tile([128, B], bfdt, tag="sc")
                        sd = wpool.tile([128, B], bfdt, tag="sd")
                        wgt = gpool.tile([128, B], bfdt, tag=f"wgt{j}")

                        x_ps = xpool.tile([128, B], dt, tag="x")

                        def YEARLY(q):
                            yf = qpool.tile([128, BQ], dt, tag="yf",
                                            name=f"yf{j}_{q}")
                            nc.tensor.matmul(yf[:, :], djy_t, zqs[q][:, :],
                                             start=True, stop=True)
                            return yf, slice(q * BQ, (q + 1) * BQ)

                        yf_early = []
                        if j > 0:
                            yf_early = [YEARLY(0), YEARLY(1)]
                        for q in range(NQ):
                            nc.tensor.matmul(x_ps[:, q * BQ:(q + 1) * BQ],
                                             djx_t, zqs[q][:, :],
                                             start=True, stop=True)
                        if j == 0:
                            yf_early = [YEARLY(0), YEARLY(1)]
                        if j > 0:
                            that_p = that_all[:, (j - 1) * 128:j * 128]
                            for q2 in range(2):
                                qs2 = slice(q2 * BQ, (q2 + 1) * BQ)
                                nc.tensor.matmul(tph0[:, qs2], that_p,
                                                 wgts[j - 1][:, qs2],
                                                 start=(j - 1 == 0),
                                                 stop=False)

                        def E(k):
                            col = k * MT + j
                            nc.scalar.activation(
                                slabs[k // 2][:, (k % 2) * B:(k % 2 + 1) * B],
                                x_ps[:, :], AF.Exp,
                                bias=pv[:, col:col + 1],
                                scale=pu[:, col:col + 1])

                        def YFpe(q):
                            """PE part: y matmul (q0/q1 pre-issued)."""
                            if q < 2:
                                return yf_early[q]
                            qs = slice(q * BQ, (q + 1) * BQ)
                            yf = qpool.tile([128, BQ], dt, tag="yf",
                                            name=f"yf{j}_{q}")
                            nc.tensor.matmul(yf[:, :], djy_t, zqs[q][:, :],
                                             start=True, stop=True)
                            return yf, qs

                        def SQY(yf, qs):
                            nc.scalar.activation(yy[:, qs], yf[:, :],
                                                 AF.Square)

                        def F3pe(yf, qs, q):
                            nc.tensor.matmul(yf[:, :], f3z_t, zqs[q][:, :],
                                             start=True, stop=False)
                            nc.tensor.matmul(yf[:, :], f3d_t,
                                             dqs[:, qs],
                                             start=False, stop=False)
                            nc.tensor.matmul(yf[:, :], f3c8_t, rhsc8[:, qs],
                                             start=False, stop=True)

                        def BASE(yf, qs):
                            nc.vector.tensor_add(ss[:, qs], xx[:, qs],
                                                 yy[:, qs])
                            nc.vector.scalar_tensor_tensor(
                                in2[:, qs], ss[:, qs], pGd[:, j:j + 1],
                                yf[:, :], AO.mult, AO.add)

                        SL = lambda p, i: slabs[p][:, i * B:(i + 1) * B]

                        # ACT queue order (strict FIFO): Sqx, E0, Sqy0,
                        # E1, Sqy1, E2, Sqy2, E3, Sqy3, E4..E7, EB0, EB1.
                        # All Sqy early so the in2 chain completes during
                        # E4..E7; EB0+EB1 after E7 cover the next tile's
                        # x-matmul chain (x_ps WAR hazard) without stalling.
                        nc.scalar.activation(xx[:, :], x_ps[:, :], AF.Square)
                        yf0, qs0 = YFpe(0)
                        E(0)
                        SQY(yf0, qs0)
                        yf1, qs1 = YFpe(1)
                        F3pe(yf0, qs0, 0)
                        E(1)
                        SQY(yf1, qs1)
                        BASE(yf0, qs0)
                        F3pe(yf1, qs1, 1)
                        E(2)
                        yf2, qs2 = YFpe(2)
                        SQY(yf2, qs2)
                        BASE(yf1, qs1)
                        F3pe(yf2, qs2, 2)
                        yf3, qs3 = YFpe(3)
                        E(3)
                        SQY(yf3, qs3)
                        BASE(yf2, qs2)
                        F3pe(yf3, qs3, 3)
                        eng23 = nc.vector if j == MT - 1 else nc.gpsimd
                        nc.vector.tensor_add(sa[:, :], SL(0, 0), SL(0, 1))
                        eng23.tensor_add(sc[:, :], SL(1, 0), SL(1, 1))
                        E(4)
                        E(5)
                        BASE(yf3, qs3)
                        eng23.tensor_add(sd[:, :], SL(2, 0), SL(2, 1))
                        nc.vector.tensor_add(sb[:, :], sa[:, :], sc[:, :])
                        E(6)
                        nc.vector.tensor_add(sa[:, :], sb[:, :], sd[:, :])
                        E(7)
                        nc.scalar.activation(EB[:, 0:3 * BQ],
                                             in2[:, 0:3 * BQ], AF.Exp)
                        nc.scalar.activation(EB[:, 3 * BQ:B],
                                             in2[:, 3 * BQ:B], AF.Exp)
                        nc.vector.tensor_add(sb[:, :], sa[:, :], SL(3, 0))
                        nc.vector.tensor_add(sa[:, :], sb[:, :], SL(3, 1))
                        nc.vector.tensor_mul(wgt[:, :], EB[:, :], sa[:, :])
                        wgts.append(wgt)

                    ocp = wpool.tile([128, B], dt, tag="ocp")
                    that_l = that_all[:, (MT - 1) * 128:MT * 128]
                    for q2 in range(2):
                        qs2 = slice(q2 * BQ, (q2 + 1) * BQ)
                        nc.tensor.matmul(tph0[:, qs2], that_l,
                                         wgts[MT - 1][:, qs2],
                                         start=False, stop=True)
                        nc.vector.tensor_copy(ocp[:, qs2], tph0[:, qs2])
                        nc.sync.dma_start(tout[:, qs2], ocp[:, qs2])

                with tc.tile_pool(name="tp1", bufs=1, space="PSUM") as tpool1:
                    tph1 = tpool1.tile([128, HB], dt, tag="tph1")
                    for q2 in range(2):
                        qg = 2 + q2
                        qs = slice(qg * BQ, (qg + 1) * BQ)
                        qs2 = slice(q2 * BQ, (q2 + 1) * BQ)
                        for j in range(MT):
                            that_j = that_all[:, j * 128:(j + 1) * 128]
                            nc.tensor.matmul(tph1[:, qs2], that_j,
                                             wgts[j][:, qs],
                                             start=(j == 0),
                                             stop=(j == MT - 1))
                        nc.vector.tensor_copy(ocp[:, qs], tph1[:, qs2])
                        nc.sync.dma_start(tout[:, qs], ocp[:, qs])

    nc.compile()
    return nc


def kernel(z_re, z_im, d_re, d_im, zj_re, zj_im, dj_re, dj_im,
           That_re, That_im, alpha, sig_par, sig_perp, _emulate=False):
    p = _prep(z_re, z_im, d_re, d_im, zj_re, zj_im, dj_re, dj_im,
              That_re, That_im, alpha, sig_par, sig_perp)
    maps = [_core_slices(p, c) for c in range(NCORES)]

    if _emulate:
        outs = [_emulate_core(m) for m in maps]
    else:
        from concourse.bass_utils import run_bass_kernel_spmd
        if "nc" not in _CACHE:
            _CACHE["nc"] = _build_bass()
        dev_maps = _device_maps(maps)
        res = run_bass_kernel_spmd(_CACHE["nc"], dev_maps,
                                   core_ids=list(range(NCORES)))
        outs = [res.results[c]["tout"] for c in range(NCORES)]

    full = np.zeros((128, B), np.float64)
    for o in outs:
        full += o.astype(np.float64)
    full = full.astype(f32).T                   # [B, 128]
    return (full[:, :S] + 1j * full[:, S:]).astype(np.complex64)


# revision 9
# speedup vs baseline: 1.1536x; 1.0215x over previous
"""CPSF fused codebook kernel for Trainium2 (8 NeuronCores, codebook-parallel).

Sharding: M (codebook, 4096) split 8 ways -> 512 entries/core; every core sees
all B=2048 queries. Host sums the 8 partial [B,S] outputs.

Math (per m-tile j of 128 codebook entries):
  x = djx^T z, y = djy^T z                      (PE, f32r)
  F3 = f3z^T z + f3d^T d + f3c8^T rhsc8        (PE, f32r/bf16; holds c_o*q0,
                                                pconst, angular dist, log-alpha)
  in2 = Gd*(x^2+y^2) + F3                      (ACT squares + DVE add/STT)
  S   = sum_k exp(u'_k x + v'_k)               (8 ACT exps -> bf16 slabs,
                                                DVE/GPSIMD add tree)
  wgt = exp(in2) * S                           (ACT exp + DVE mul, bf16)
  T  += That^T wgt                             (PE, bf16)

ACT is the bottleneck engine (9 exps + 2 squares per (m-tile, query) are
algorithmically required); emission order keeps its queue streaming:
Sqx, E0, Sqy0, E1, Sqy1, E2, Sqy2, E3, E4, Sqy3, E5..E7, EB per tile.
"""

import numpy as np
import ml_dtypes

B, M, N, S, K = 2048, 4096, 64, 64, 8
EPS = 1e-3
NCORES = 8
ML = M // NCORES          # 512 codebook entries per core
MT = ML // 128            # 4 m-tiles per core
NQ = 4                    # b-quarters (PSUM-sized chunks of 512)
BQ = B // NQ              # 512
f32 = np.float32
bf16 = ml_dtypes.bfloat16

_CACHE = {}


def _bf16_rt(x):
    u = np.ascontiguousarray(x, np.float32).view(np.uint32)
    r = (u + 0x7FFF + ((u >> 16) & 1)) & 0xFFFF0000
    return r.view(np.float32).astype(np.float64)


def _prep(z_re, z_im, d_re, d_im, zj_re, zj_im, dj_re, dj_im,
          That_re, That_im, alpha, sig_par, sig_perp):
    """Host-side packing: fp64 exact, cast to fp32/bf16 at the end."""
    x64 = lambda a: np.asarray(a, np.float64)
    zr, zi, dr, di = map(x64, (z_re, z_im, d_re, d_im))
    zjr, zji, djr, dji = map(x64, (zj_re, zj_im, dj_re, dj_im))

    tgl, wgl = np.polynomial.legendre.leggauss(K)
    t = (0.5 * (tgl + 1.0)).astype(f32).astype(np.float64)
    wq = (0.5 * wgl).astype(f32).astype(np.float64)

    dd2 = (djr**2 + dji**2).sum(-1)                          # [M]
    c_re = (djr * zjr + dji * zji).sum(-1)
    c_im = (djr * zji - dji * zjr).sum(-1)
    sp2 = x64(sig_par)**2 + EPS
    so2 = x64(sig_perp)**2 + EPS
    G = -0.5 / sp2
    c_o = -0.5 / so2
    Gd = G - c_o
    umid = -G * dd2
    lnal = np.log(np.maximum(x64(alpha), 1e-38))
    nzj = (zjr**2 + zji**2).sum(-1)
    nz = (zr**2 + zi**2).sum(-1)                             # [B]
    nd = (dr**2 + di**2).sum(-1)

    u = np.stack([-2.0 * G * t[k] * dd2 for k in range(K)])  # [K,M]
    up = u - umid[None, :]
    vp = np.stack([np.log(wq[k]) + G * (t[k] * dd2)**2 - up[k] * c_re
                   for k in range(K)])

    djx = np.concatenate([djr.T, dji.T], 0)                  # [128, M]
    djy = np.concatenate([-dji.T, djr.T], 0)
    f3z = ((-2.0 * c_o) * np.concatenate([zjr.T, zji.T], 0)
           + (-2.0 * Gd * c_re + umid) * djx
           + (-2.0 * Gd * c_im) * djy)
    f3d = (-2.0 * c_o) * np.concatenate([djr.T, dji.T], 0)
    const0 = (c_o * (nzj + dd2) + Gd * (c_re**2 + c_im**2)
              + lnal - umid * c_re)

    # f3c8/rhsc8: rank-8 bf16-exact update for co*(nz+nd) + pconst
    co_h = _bf16_rt(c_o)
    co_l = c_o - co_h
    pc_h = _bf16_rt(const0)
    pc_l = const0 - pc_h
    nz_h = _bf16_rt(nz)
    nz_l = nz - nz_h
    nd_h = _bf16_rt(nd)
    nd_l = nd - nd_h
    ones = np.ones_like(nz)
    f3c8 = np.stack([co_h, co_h, co_l, co_h, co_h, co_l, pc_h, pc_l])
    rhsc8 = np.stack([nz_h, nz_l, nz_h, nd_h, nd_l, nd_h, ones, ones])
    that2 = np.concatenate([x64(That_re), x64(That_im)], 1)  # [M, 128]

    # pk: per m-tile block [djx | djy | f3z], each 128 cols (f32r path)
    nt = M // 128
    pk = np.empty((128, nt * 384), np.float64)
    for j in range(nt):
        ms = slice(j * 128, (j + 1) * 128)
        pk[:, j * 384 + 0:j * 384 + 128] = djx[:, ms]
        pk[:, j * 384 + 128:j * 384 + 256] = djy[:, ms]
        pk[:, j * 384 + 256:j * 384 + 384] = f3z[:, ms]

    pGd = Gd.reshape(nt, 128).T                              # [128, nt]
    pu = up.reshape(K, nt, 128).transpose(2, 0, 1).reshape(128, K * nt)
    pv = vp.reshape(K, nt, 128).transpose(2, 0, 1).reshape(128, K * nt)

    c = lambda a: np.ascontiguousarray(a, dtype=f32)
    cb = lambda a: np.ascontiguousarray(np.asarray(a, np.float32), dtype=bf16)
    return dict(pk=c(pk), f3c8=cb(f3c8), rhsc8=cb(rhsc8), that2=cb(that2),
                f3db=cb(f3d), pGd=c(pGd), pu=c(pu), pv=c(pv),
                zst=c(np.concatenate([zr.T, zi.T], 0)),
                dstb=cb(np.concatenate([dr.T, di.T], 0)))


def _core_slices(p, cid):
    """Per-core in_map from the full packed arrays (m-sharded)."""
    jt = slice(cid * MT * 384, (cid + 1) * MT * 384)         # pk cols
    ms = slice(cid * ML, (cid + 1) * ML)
    jc = slice(cid * MT, (cid + 1) * MT)
    kc = np.concatenate([np.arange(k * (M // 128) + cid * MT,
                                   k * (M // 128) + (cid + 1) * MT)
                         for k in range(K)])
    cc = np.ascontiguousarray
    return {"pk": cc(p["pk"][:, jt]), "f3c8": cc(p["f3c8"][:, ms]),
            "that2": cc(p["that2"][ms, :]), "pGd": cc(p["pGd"][:, jc]),
            "f3db": cc(p["f3db"][:, ms]),
            "pu": cc(p["pu"][:, kc]), "pv": cc(p["pv"][:, kc]),
            "zst": p["zst"], "dstb": p["dstb"], "rhsc8": p["rhsc8"]}


def _device_maps(maps):
    dev_maps = []
    for m in maps:
        dm = {k: m[k] for k in ("pk", "f3c8", "that2", "zst", "dstb",
                                "rhsc8", "f3db")}
        dm["params"] = np.ascontiguousarray(np.concatenate(
            [m["pGd"], m["pu"], m["pv"]], axis=1))
        dev_maps.append(dm)
    return dev_maps


def _emulate_core(m):
    """Numpy emulation of one core's device program."""
    zst = np.asarray(m["zst"], f32)
    dstb = np.asarray(m["dstb"], np.float32)
    rhsc8 = np.asarray(m["rhsc8"], np.float32)
    that2 = np.asarray(m["that2"], np.float32)
    f3db = np.asarray(m["f3db"], np.float32)
    t_acc = np.zeros((128, 2048), f32)
    for j in range(MT):
        pkj = m["pk"][:, j * 384:(j + 1) * 384]
        djx_t, djy_t, f3z_t = pkj[:, 0:128], pkj[:, 128:256], pkj[:, 256:384]
        f3d_t = f3db[:, j * 128:(j + 1) * 128]
        x = (djx_t.T @ zst).astype(f32)
        y = (djy_t.T @ zst).astype(f32)
        F3 = (f3z_t.T @ zst + f3d_t.T @ dstb
              + m["f3c8"][:, j * 128:(j + 1) * 128].astype(np.float32).T
              @ rhsc8).astype(f32)
        s = (x * x + y * y).astype(f32)
        in2 = (s * m["pGd"][:, j:j + 1] + F3).astype(f32)
        Ssum = np.zeros_like(x)
        for k in range(K):
            col = k * MT + j
            arg = (x * m["pu"][:, col:col + 1] + m["pv"][:, col:col + 1]).astype(f32)
            Ssum = (Ssum + np.exp(arg, dtype=f32)).astype(f32)
        wgt = (np.exp(in2, dtype=f32) * Ssum).astype(f32)
        that_t = that2[j * 128:(j + 1) * 128, :]
        t_acc += (that_t.T @ wgt).astype(f32)
    return t_acc


def _build_bass():
    import concourse.bacc as bacc
    import concourse.mybir as mybir
    from concourse import tile
    import concourse.hw_specs as _hwsp
    # The scheduler's cost sim models f32r matmuls at 1 cyc/row; real HW
    # streams them at ~2 cyc/row. Halving the modeled PE clock makes the
    # static schedule place ACT filler ops into the real matmul shadows.
    _hwsp.TRN2Spec.PE_CYCLE = 1e9 / 1.2e9

    dt = mybir.dt.float32
    f32r = mybir.dt.float32r
    bfdt = mybir.dt.bfloat16
    AF = mybir.ActivationFunctionType
    AO = mybir.AluOpType
    nc = bacc.Bacc("TRN2", target_bir_lowering=False, debug=False)

    dram = {}
    for name, shape, dty in [("zst", [128, B], f32r),
                             ("dstb", [128, B], bfdt),
                             ("rhsc8", [8, B], bfdt),
                             ("pk", [128, MT * 384], f32r),
                             ("f3c8", [8, ML], bfdt),
                             ("f3db", [128, ML], bfdt),
                             ("that2", [ML, 128], bfdt),
                             ("params", [128, MT + 2 * K * MT], dt)]:
        dram[name] = nc.dram_tensor(name, shape, dty, kind="ExternalInput")
    tout = nc.dram_tensor("tout", [128, B], dt, kind="ExternalOutput")

    HB = B // 2
    with tile.TileContext(nc) as tc:
        with tc.tile_pool(name="const", bufs=1) as cpool:
            params = cpool.tile([128, MT + 2 * K * MT], dt)
            pGd = params[:, 0:MT]
            pu = params[:, MT:MT + K * MT]
            pv = params[:, MT + K * MT:MT + 2 * K * MT]
            rhsc8 = cpool.tile([8, B], bfdt)
            that_all = cpool.tile([128, MT * 128], bfdt)
            f3c8_all = cpool.tile([8, ML], bfdt)
            f3db_all = cpool.tile([128, ML], bfdt)
            warm = cpool.tile([128, 8], dt)
            zqs = [cpool.tile([128, BQ], f32r, name=f"zq{i}") for i in range(NQ)]
            dqs = cpool.tile([128, B], bfdt)

            with (
                tc.tile_pool(name="lhs", bufs=2) as lpool,
                tc.tile_pool(name="work", bufs=1) as wpool,
                tc.tile_pool(name="eslab", bufs=2) as epool,
                tc.tile_pool(name="wgtp", bufs=1) as gpool,
            ):
                # Critical-path DMAs (zq, pk, params) on sync, in
                # priority order.  Bulk DMAs go on the gpsimd queue but are
                # gated behind zq3's arrival by a tiny gpsimd copy, so they
                # don't steal HBM bandwidth from the startup critical path.
                pk0x = lpool.tile([128, 128], f32r, tag="pkx")
                pk0 = lpool.tile([128, 384], f32r, tag="pk")
                nc.sync.dma_start(pk0x[:, :], dram["pk"][:, 0:128])
                nc.sync.dma_start(zqs[0][:, :], dram["zst"][:, 0:BQ])
                # warm the exp table as soon as the first DMA lands
                nc.scalar.activation(warm[:, :], pk0x[:, 0:8], AF.Exp)
                nc.scalar.dma_start(zqs[1][:, :], dram["zst"][:, BQ:2 * BQ])
                nc.gpsimd.dma_start(zqs[2][:, :], dram["zst"][:, 2 * BQ:3 * BQ])
                nc.scalar.dma_start(zqs[3][:, :], dram["zst"][:, 3 * BQ:4 * BQ])
                nc.sync.dma_start(pk0[:, :], dram["pk"][:, 0:384])
                nc.gpsimd.dma_start(params[:, :], dram["params"][:, :])
                nc.gpsimd.dma_start(f3db_all[:, :], dram["f3db"][:, :])
                nc.gpsimd.dma_start(dqs[:, 0:BQ], dram["dstb"][:, 0:BQ])
                nc.gpsimd.dma_start(rhsc8[:, :], dram["rhsc8"][:, :])
                nc.gpsimd.dma_start(f3c8_all[:, :], dram["f3c8"][:, :])
                nc.gpsimd.dma_start(dqs[:, BQ:B], dram["dstb"][:, BQ:B])
                nc.gpsimd.dma_start(
                    that_all[:, :].rearrange("p (j c) -> p j c", j=MT),
                    dram["that2"][:, :].rearrange("(j p) c -> p j c", p=128))
                wgts = []
                with (
                    tc.tile_pool(name="xps", bufs=1, space="PSUM") as xpool,
                    tc.tile_pool(name="yf", bufs=2, space="PSUM") as qpool,
                    tc.tile_pool(name="tp0", bufs=1, space="PSUM") as tpool0,
                ):
                    tph0 = tpool0.tile([128, HB], dt, tag="tph0")
                    for j in range(MT):
                        if j == 0:
                            pk_t, pkx_t = pk0, pk0x
                        else:
                            pk_t = lpool.tile([128, 384], f32r, tag="pk")
                            pkx_t = lpool.tile([128, 128], f32r, tag="pkx")
                            nc.sync.dma_start(
                                pkx_t[:, :],
                                dram["pk"][:, j * 384:j * 384 + 128])
                            nc.sync.dma_start(
                                pk_t[:, :],
                                dram["pk"][:, j * 384:(j + 1) * 384])
                        djx_t = pkx_t[:, :]
                        djy_t = pk_t[:, 128:256]
                        f3z_t = pk_t[:, 256:384]
                        f3c8_t = f3c8_all[:, j * 128:(j + 1) * 128]
                        f3d_t = f3db_all[:, j * 128:(j + 1) * 128]

                        xx = wpool.tile([128, B], dt, tag="xx")
                        yy = wpool.tile([128, B], dt, tag="yy")
                        ss = wpool.tile([128, B], dt, tag="ss")
                        in2 = wpool.tile([128, B], dt, tag="in2")
                        EB = wpool.tile([128, B], bfdt, tag="EB")
                        slabs = [epool.tile([128, 2 * B], bfdt, tag=f"esl{p}",
                                            name=f"esl{p}_{j}")
                                 for p in range(4)]
                        sa = wpool.tile([128, B], bfdt, tag="sa")
                        sb = wpool.tile([128, B], bfdt, tag="sb")
                        sc = wpool.tile([128, B], bfdt, tag="sc")
                        sd = wpool.tile([128, B], bfdt, tag="sd")
                        wgt = gpool.tile([128, B], bfdt, tag=f"wgt{j}")

                        x_ps = xpool.tile([128, B], dt, tag="x")

                        def YEARLY(q):
                            yf = qpool.tile([128, BQ], dt, tag="yf",
                                            name=f"yf{j}_{q}")
                            nc.tensor.matmul(yf[:, :], djy_t, zqs[q][:, :],
                                             start=True, stop=True)
                            return yf, slice(q * BQ, (q + 1) * BQ)

                        yf_early = []
                        if j > 0:
                            yf_early = [YEARLY(0), YEARLY(1)]
                        for q in range(NQ):
                            nc.tensor.matmul(x_ps[:, q * BQ:(q + 1) * BQ],
                                             djx_t, zqs[q][:, :],
                                             start=True, stop=True)
                        if j == 0:
                            yf_early = [YEARLY(0), YEARLY(1)]
                        if j > 0:
                            that_p = that_all[:, (j - 1) * 128:j * 128]
                            for q2 in range(2):
                                qs2 = slice(q2 * BQ, (q2 + 1) * BQ)
                                nc.tensor.matmul(tph0[:, qs2], that_p,
                                                 wgts[j - 1][:, qs2],
                                                 start=(j - 1 == 0),
                                                 stop=False)

                        def E(k):
                            col = k * MT + j
                            nc.scalar.activation(
                                slabs[k // 2][:, (k % 2) * B:(k % 2 + 1) * B],
                                x_ps[:, :], AF.Exp,
                                bias=pv[:, col:col + 1],
                                scale=pu[:, col:col + 1])

                        def YFpe(q):
                            """PE part: y matmul (q0/q1 pre-issued)."""
                            if q < 2:
                                return yf_early[q]
                            qs = slice(q * BQ, (q + 1) * BQ)
                            yf = qpool.tile([128, BQ], dt, tag="yf",
                                            name=f"yf{j}_{q}")
                            nc.tensor.matmul(yf[:, :], djy_t, zqs[q][:, :],
                                             start=True, stop=True)
                            return yf, qs

                        def SQY(yf, qs):
                            nc.scalar.activation(yy[:, qs], yf[:, :],
                                                 AF.Square)

                        def F3pe(yf, qs, q):
                            nc.tensor.matmul(yf[:, :], f3z_t, zqs[q][:, :],
                                             start=True, stop=False)
                            nc.tensor.matmul(yf[:, :], f3d_t,
                                             dqs[:, qs],
                                             start=False, stop=False)
                            nc.tensor.matmul(yf[:, :], f3c8_t, rhsc8[:, qs],
                                             start=False, stop=True)

                        def BASE(yf, qs):
                            nc.vector.tensor_add(ss[:, qs], xx[:, qs],
                                                 yy[:, qs])
                            nc.vector.scalar_tensor_tensor(
                                in2[:, qs], ss[:, qs], pGd[:, j:j + 1],
                                yf[:, :], AO.mult, AO.add)

                        SL = lambda p, i: slabs[p][:, i * B:(i + 1) * B]

                        # ACT queue order (strict FIFO): Sqx, E0, Sqy0,
                        # E1, Sqy1, E2, Sqy2, E3, Sqy3, E4..E7, EB0, EB1.
                        # All Sqy early so the in2 chain completes during
                        # E4..E7; EB0+EB1 after E7 cover the next tile's
                        # x-matmul chain (x_ps WAR hazard) without stalling.
                        nc.scalar.activation(xx[:, :], x_ps[:, :], AF.Square)
                        yf0, qs0 = YFpe(0)
                        E(0)
                        SQY(yf0, qs0)
                        yf1, qs1 = YFpe(1)
                        F3pe(yf0, qs0, 0)
                        E(1)
                        SQY(yf1, qs1)
                        BASE(yf0, qs0)
                        F3pe(yf1, qs1, 1)
                        E(2)
                        yf2, qs2 = YFpe(2)
                        SQY(yf2, qs2)
                        BASE(yf1, qs1)
                        F3pe(yf2, qs2, 2)
                        yf3, qs3 = YFpe(3)
                        E(3)
                        SQY(yf3, qs3)
                        BASE(yf2, qs2)
                        F3pe(yf3, qs3, 3)
                        eng23 = nc.vector if j == MT - 1 else nc.gpsimd
                        nc.vector.tensor_add(sa[:, :], SL(0, 0), SL(0, 1))
                        eng23.tensor_add(sc[:, :], SL(1, 0), SL(1, 1))
                        E(4)
                        E(5)
                        BASE(yf3, qs3)
                        eng23.tensor_add(sd[:, :], SL(2, 0), SL(2, 1))
                        nc.vector.tensor_add(sb[:, :], sa[:, :], sc[:, :])
                        E(6)
                        nc.vector.tensor_add(sa[:, :], sb[:, :], sd[:, :])
                        E(7)
                        nc.scalar.activation(EB[:, 0:3 * BQ],
                                             in2[:, 0:3 * BQ], AF.Exp)
                        nc.scalar.activation(EB[:, 3 * BQ:B],
                                             in2[:, 3 * BQ:B], AF.Exp)
                        nc.vector.tensor_add(sb[:, :], sa[:, :], SL(3, 0))
                        if j < MT - 1:
                            nc.vector.tensor_add(sa[:, :], sb[:, :],
                                                 SL(3, 1))
                            nc.vector.tensor_mul(wgt[:, :], EB[:, :],
                                                 sa[:, :])
                        else:
                            # split so the That tail can start on half 1
                            for hs in (slice(0, HB), slice(HB, B)):
                                nc.vector.tensor_add(sa[:, hs], sb[:, hs],
                                                     SL(3, 1)[:, hs])
                                nc.vector.tensor_mul(wgt[:, hs], EB[:, hs],
                                                     sa[:, hs])
                        wgts.append(wgt)

                    ocp = wpool.tile([128, B], dt, tag="ocp")
                    that_l = that_all[:, (MT - 1) * 128:MT * 128]
                    for q2 in range(2):
                        qs2 = slice(q2 * BQ, (q2 + 1) * BQ)
                        nc.tensor.matmul(tph0[:, qs2], that_l,
                                         wgts[MT - 1][:, qs2],
                                         start=False, stop=True)
                        nc.vector.tensor_copy(ocp[:, qs2], tph0[:, qs2])
                        nc.sync.dma_start(tout[:, qs2], ocp[:, qs2])

                with tc.tile_pool(name="tp1", bufs=1, space="PSUM") as tpool1:
                    tph1 = tpool1.tile([128, HB], dt, tag="tph1")
                    for q2 in range(2):
                        qg = 2 + q2
                        qs = slice(qg * BQ, (qg + 1) * BQ)
                        qs2 = slice(q2 * BQ, (q2 + 1) * BQ)
                        for j in range(MT - 1):
                            that_j = that_all[:, j * 128:(j + 1) * 128]
                            nc.tensor.matmul(tph1[:, qs2], that_j,
                                             wgts[j][:, qs],
                                             start=(j == 0), stop=False)
                    for q2 in range(2):
                        qg = 2 + q2
                        qs = slice(qg * BQ, (qg + 1) * BQ)
                        qs2 = slice(q2 * BQ, (q2 + 1) * BQ)
                        that_j = that_all[:, (MT - 1) * 128:MT * 128]
                        nc.tensor.matmul(tph1[:, qs2], that_j,
                                         wgts[MT - 1][:, qs],
                                         start=False, stop=True)
                        nc.vector.tensor_copy(ocp[:, qs], tph1[:, qs2])
                        nc.sync.dma_start(tout[:, qs], ocp[:, qs])

    nc.compile()
    return nc


def kernel(z_re, z_im, d_re, d_im, zj_re, zj_im, dj_re, dj_im,
           That_re, That_im, alpha, sig_par, sig_perp, _emulate=False):
    p = _prep(z_re, z_im, d_re, d_im, zj_re, zj_im, dj_re, dj_im,
              That_re, That_im, alpha, sig_par, sig_perp)
    maps = [_core_slices(p, c) for c in range(NCORES)]

    if _emulate:
        outs = [_emulate_core(m) for m in maps]
    else:
        from concourse.bass_utils import run_bass_kernel_spmd
        if "nc" not in _CACHE:
            _CACHE["nc"] = _build_bass()
        dev_maps = _device_maps(maps)
        res = run_bass_kernel_spmd(_CACHE["nc"], dev_maps,
                                   core_ids=list(range(NCORES)))
        outs = [res.results[c]["tout"] for c in range(NCORES)]

    full = np.zeros((128, B), np.float64)
    for o in outs:
        full += o.astype(np.float64)
    full = full.astype(f32).T                   # [B, 128]
    return (full[:, :S] + 1j * full[:, S:]).astype(np.complex64)
